# revision 1
# baseline (speedup 1.0000x reference)
"""Multi-head self-attention (B=2, T=2048, D=2048, H=16, RoPE, causal)
as a Bass/Tile kernel running SPMD on 8 trn2 NeuronCores.

Sharding: tensor-parallel over heads (2 heads per core). Each core
computes its heads' Q/K/V projections, RoPE, causal attention, and a
partial out-projection over its 256 feature columns; the host sums the
8 partial outputs (all-reduce equivalent).

Dataflow (per core, per batch):
  - x streamed per 512-wide t-block ([128, 16, 512] SBUF tiles, 4 tags);
    the first block's DMA is interleaved per-contraction-chunk with the
    weight loads so the PE starts ~2us in.
  - Q/K projections in "T-layout" (feature dim on partitions, time on
    free); RoPE rotate-half via two SBUF->SBUF partition-swap DMAs
    (sign folded into the sin table), all-bf16 combine on DVE (2x mode).
  - V projected directly in natural layout ([tk, d]): lhsT = x chunk,
    rhs = Wv slice -- no PE transposes.
  - scores computed transposed: S^T[tk, tq] per (key-chunk, q-group).
    Chunks are narrowed to the causal region (exact 136-block lower
    triangle, no fully-masked work); only the diagonal 128x128 block
    gets a mask add. The two heads' chunk streams are interleaved so
    the PE always has ~1.3us of work while exp round-trips through
    DVE/Act. Z row sums via a [128,1] ones matmul accumulated in PSUM.
  - normalization trails each q-group: po -> oT (unnormalized cast),
    1/Z table via DVE reciprocal, then a ones-row broadcast matmul
    (riding the po PSUM slots between groups) + in-place DVE multiply.
  - out-projection accumulates the two head-chunks in PSUM; partial
    result cast to f16 and DMA'd out; host sums partials across cores.
"""

import sys

sys.path.insert(0, "/opt/trn_rl_repo")

import ml_dtypes
import numpy as np

import concourse.bass as bass
import concourse.mybir as mybir
import concourse.tile as tile
from concourse.bass_utils import run_bass_kernel_spmd


def _legalize_waits(nc):
    """Walrus codegen rejects >2 sync waits on DMA/matmul/nop-class
    instructions, and Tile's pool-recycle waits bypass its own elision.
    Spill excess waits (>1) onto freshly inserted same-engine NoOps
    placed immediately before the offending instruction (sound w.r.t.
    per-engine program order)."""
    spill_id = [0]
    for bb in nc.m.functions[0].blocks:
        new_insts = []
        for inst in bb.instructions:
            si = getattr(inst, "sync_info", None)
            if si is None or not si.on_wait:
                new_insts.append(inst)
                continue
            eng = getattr(inst, "engine", None)
            kept = list(si.on_wait)
            if len(kept) > 1 and eng is not None:
                excess, kept = kept[:-1], kept[-1:]
                for w in excess:
                    spill_id[0] += 1
                    nop = mybir.InstNoOp(
                        name=f"I-wspill-{spill_id[0]}",
                        ins=[],
                        outs=[],
                        engine=eng,
                    )
                    nop.sync_info = mybir.SyncInfo(on_wait=[w], on_update=[])
                    new_insts.append(nop)
            if len(kept) != len(si.on_wait):
                si.on_wait[:] = kept
            new_insts.append(inst)
        if len(new_insts) != len(bb.instructions):
            bb.instructions[:] = new_insts


_PHASE_MARKS = []  # (phase_label, last_inst_index_before_phase) - profiling aid


def _mark(nc, label):
    n = -1
    for fn in nc.m.functions:
        for bb in fn.blocks:
            for ins in bb.instructions:
                if ins.name.startswith("I-"):
                    try:
                        n = max(n, int(ins.name[2:]))
                    except ValueError:
                        pass
    _PHASE_MARKS.append((label, n))


B, T, D, H, HD = 2, 2048, 2048, 16, 128
NCORES = 8
HPC = H // NCORES            # heads per core = 2
M_PC = HPC * HD              # per-core feature slice = 256
BT = B * T                   # 4096
SCALE = HD ** -0.5
ROPE_THETA = 10000.0

F32 = mybir.dt.float32
F16 = mybir.dt.float16
BF16 = mybir.dt.bfloat16
BF16_NP = ml_dtypes.bfloat16

TB = 512                     # t-block for projections / q-groups
NTB_B = T // TB              # 4 t-blocks per batch
NMC = D // 128               # 16 contraction chunks
NKC = T // 128               # 16 key chunks per batch
JPG = TB // 128              # key chunks per q-group width = 4

Copy = mybir.ActivationFunctionType.Copy
Exp = mybir.ActivationFunctionType.Exp


def build_program():
    nc = bass.Bass()

    xT_d = nc.declare_dram_parameter("xT", [D, BT], BF16, isOutput=False)
    negm_d = nc.declare_dram_parameter("negmM", [128, 128], F32, isOutput=False)
    # wq and wk concatenated so one DMA covers both (halves SP-seq time
    # on the critical startup path)
    wqk_d = nc.declare_dram_parameter(
        "wqkT", [D, 2 * M_PC], BF16, isOutput=False
    )
    wv_d = nc.declare_dram_parameter("wvT", [D, M_PC], BF16, isOutput=False)
    wo_d = nc.declare_dram_parameter("woT", [M_PC, D], BF16, isOutput=False)
    cos_d = nc.declare_dram_parameter("cosT", [HD, T], BF16, isOutput=False)
    sinh_d = nc.declare_dram_parameter("sinhT", [HD, T], BF16, isOutput=False)
    out_d = nc.declare_dram_parameter("partialT", [D, BT], F16, isOutput=True)

    xT_v = xT_d.rearrange("(c p) t -> p c t", p=128)      # [128, 16, BT]
    wqk_v = wqk_d.rearrange("(c p) n -> p c n", p=128)    # [128, 16, 512]
    wv_v = wv_d.rearrange("(c p) n -> p c n", p=128)
    wo_v = wo_d.rearrange("(c p) n -> p c n", p=128)      # [128, 2, 2048]
    out_v = out_d.rearrange("(c p) t -> p c t", p=128)    # [128, 16, BT]

    with tile.TileContext(nc) as tc:
        with (
            tc.tile_pool(name="wpool", bufs=1) as wpool,
            tc.tile_pool(name="xp", bufs=1) as xp,
            tc.tile_pool(name="big", bufs=1) as big,
            tc.tile_pool(name="rp", bufs=2) as rp,
            tc.tile_pool(name="attn_sb", bufs=8) as asb,
            tc.tile_pool(name="fs_sb", bufs=3) as fsb,
        ):
            # ---- weights + first x block, interleaved in graduated mc
            # groups (fast pipeline fill, then few big SP-cheap DMAs) ----
            wqk_sb = wpool.tile([128, NMC, 2 * M_PC], BF16, tag="wqk")
            wv_sb = wpool.tile([128, NMC, M_PC], BF16, tag="wv")
            x_tiles = {}
            xt0 = xp.tile([128, NMC, TB], BF16, tag="x0", name="x_b0_t0")
            x_tiles[(0, 0)] = xt0
            for lo, hi in ((0, 1), (1, 2), (2, 3), (3, 4), (4, 6), (6, 8),
                           (8, 10), (10, 12), (12, 14), (14, 16)):
                nc.sync.dma_start(
                    out=wqk_sb[:, lo:hi, :], in_=wqk_v[:, lo:hi, :]
                )
                # first x chunk rides the idle DVE queue, in parallel with
                # SP's weight DMA, to cut the cold-start latency
                eng = nc.scalar if lo == 0 else nc.sync
                eng.dma_start(
                    out=xt0[:, lo:hi, :], in_=xT_v[:, lo:hi, 0:TB]
                )

            cos_sb = wpool.tile([128, T], BF16, tag="cos")
            sinh_sb = wpool.tile([128, T], BF16, tag="sinh")
            nc.sync.dma_start(out=cos_sb[:, 0:TB], in_=cos_d[:, 0:TB])
            nc.sync.dma_start(out=sinh_sb[:, 0:TB], in_=sinh_d[:, 0:TB])

            def load_x(b, tb):
                t = xp.tile(
                    [128, NMC, TB], BF16, tag=f"x{tb}", name=f"x_b{b}_t{tb}"
                )
                x_tiles[(b, tb)] = t
                lo = b * T + tb * TB
                for m0 in range(0, NMC, 4):
                    nc.sync.dma_start(
                        out=t[:, m0 : m0 + 4, :],
                        in_=xT_v[:, m0 : m0 + 4, lo : lo + TB],
                    )

            # wv rides alongside tb0's V matmuls; x block 1 follows
            for m0 in range(0, NMC, 4):
                nc.sync.dma_start(
                    out=wv_sb[:, m0 : m0 + 4, :], in_=wv_v[:, m0 : m0 + 4, :]
                )
            load_x(0, 1)
            nc.sync.dma_start(out=cos_sb[:, TB:], in_=cos_d[:, TB:])
            nc.sync.dma_start(out=sinh_sb[:, TB:], in_=sinh_d[:, TB:])
            negm = wpool.tile([128, 128], F32, tag="negm")
            nc.sync.dma_start(out=negm, in_=negm_d[:, :])
            ones_col = wpool.tile([128, 1], BF16, tag="ones_c")
            nc.vector.memset(ones_col, 1.0)
            ones_row = wpool.tile([1, 128], BF16, tag="ones_r")
            nc.vector.memset(ones_row, 1.0)
            # 1/Z table: [1, HPC*T], column h*T + t (kept on partition 0)
            zrs_tab = wpool.tile([1, HPC * T], BF16, tag="zrs")

            for tb in range(2, NTB_B):
                load_x(0, tb)

            wo_sb = wpool.tile([128, HPC, D], BF16, tag="wo")
            nc.sync.dma_start(out=wo_sb, in_=wo_v)

            for b in range(B):
                t0 = b * T  # global t offset of this batch
                _mark(nc, f"b{b}_proj")

                # persistent per-batch tensors (slots reused across b)
                qT = big.tile([128, HPC, T], BF16, tag="qT")   # [hd, h, t]
                kT = big.tile([128, HPC, T], BF16, tag="kT")
                vN = big.tile([128, NKC, M_PC], BF16, tag="vN")  # [tk, j, n]
                oT = big.tile([128, HPC, T], BF16, tag="oT")   # attn out

                # ---------------- projections + RoPE ----------------
                warm = []  # pre-emitted attention chunks
                with tc.tile_pool(name="qk_ps", bufs=1, space="PSUM") as qkp, \
                     tc.tile_pool(name="v_ps", bufs=1, space="PSUM") as vps:
                    for tb in range(NTB_B):
                        xt = x_tiles[(b, tb)]
                        ts_l = slice(tb * TB, (tb + 1) * TB)   # in-batch
                        ps = {}
                        for h in range(HPC):
                            for nm in ("q", "k"):
                                ps[nm, h] = qkp.tile(
                                    [128, TB], F32, tag=f"{nm}{h}",
                                    name=f"ps_{nm}{h}",
                                )
                        for mc in range(NMC):
                            for h in range(HPC):
                                for ni, nm in ((0, "q"), (1, "k")):
                                    hs = slice(
                                        ni * M_PC + h * HD,
                                        ni * M_PC + (h + 1) * HD,
                                    )
                                    nc.tensor.matmul(
                                        ps[nm, h],
                                        lhsT=wqk_sb[:, mc, hs],
                                        rhs=xt[:, mc, :],
                                        start=(mc == 0),
                                        stop=(mc == NMC - 1),
                                    )
                        # raw bf16 casts early on Act (deps ready now)
                        raws = {}
                        for nm in ("q", "k"):
                            for h in range(HPC):
                                raw = rp.tile(
                                    [128, TB], BF16, tag=f"raw{nm}{h}"
                                )
                                nc.scalar.activation(raw, ps[nm, h], Copy)
                                raws[nm, h] = raw
                        # RoPE before V: the qk/sw PSUM banks drain while the
                        # PE streams V matmuls, so the next phase's bank-WAR
                        # waits resolve before the PE gets there
                        for nm, dest in (("q", qT), ("k", kT)):
                            for h in range(HPC):
                                raw = raws[nm, h]
                                swb = rp.tile([128, TB], BF16, tag="swb")
                                nc.sync.dma_start(
                                    out=swb[0:64, :], in_=raw[64:128, :]
                                )
                                nc.sync.dma_start(
                                    out=swb[64:128, :], in_=raw[0:64, :]
                                )
                                t2 = rp.tile([128, TB], BF16, tag="t2")
                                nc.vector.tensor_mul(t2, swb, sinh_sb[:, ts_l])
                                t1 = rp.tile([128, TB], BF16, tag="t1")
                                nc.vector.tensor_mul(t1, raw, cos_sb[:, ts_l])
                                nc.vector.tensor_add(dest[:, h, ts_l], t1, t2)
                        # V directly in natural layout: per 128-wide tk chunk
                        def v_chunk(s):
                            j = tb * JPG + s
                            pv = vps.tile(
                                [128, M_PC], F32, tag=f"v{s % 2}",
                                name=f"ps_v{s % 2}",
                            )
                            for mc in range(NMC):
                                nc.tensor.matmul(
                                    pv,
                                    lhsT=xt[:, mc, s * 128 : (s + 1) * 128],
                                    rhs=wv_sb[:, mc, :],
                                    start=(mc == 0),
                                    stop=(mc == NMC - 1),
                                )
                            nc.scalar.activation(vN[:, j, :], pv, Copy)

                        for s in range(JPG):
                            v_chunk(s)
                            # between tb3's V chunks, warm up attention: the
                            # (qg0, j0) score pair + exp run in drained qk
                            # PSUM slots, their latency hidden by V matmuls,
                            # so the attention loop starts with a full
                            # pipeline instead of a fill bubble
                            if tb == NTB_B - 1 and s in (1, 2):
                                # chunk (0,0) after V s1 in the q-tag slots,
                                # chunk (0,1) after V s2 in the k-tag slots:
                                # the attention loop starts two chunks deep,
                                # matching the Z trail exactly
                                wj = 0 if s == 1 else 1
                                woff = 0 if wj == 0 else 128
                                wtag = "q" if s == 1 else "k"
                                wcur = {}
                                for h in range(HPC):
                                    wst = qkp.tile(
                                        [128, TB], F32,
                                        tag=f"{wtag}{h}", name="warm_st",
                                    )
                                    nc.tensor.matmul(
                                        wst[:, woff:],
                                        lhsT=kT[:, h, wj * 128 : (wj + 1) * 128],
                                        rhs=qT[:, h, woff:TB],
                                        start=True,
                                        stop=True,
                                    )
                                    nc.vector.tensor_add(
                                        wst[:, woff : woff + 128],
                                        wst[:, woff : woff + 128],
                                        negm,
                                    )
                                    wpt = asb.tile(
                                        [128, TB], BF16, tag="pt",
                                        name="warm_pt",
                                    )
                                    nc.scalar.activation(
                                        wpt[:, woff:], wst[:, woff:],
                                        Exp, scale=SCALE,
                                    )
                                    wcur[h] = (wpt, woff)
                                warm.append([0, wj, wcur, False])

                if b == 0:
                    # b1's x loads: emitted here so they ride the SP queue
                    # ahead of b0's output stores
                    for tb in range(NTB_B):
                        load_x(1, tb)

                _mark(nc, f"b{b}_attn")
                # ---------------- attention ----------------
                # heads interleaved per key-chunk (both heads' score tiles
                # merged into one 2-bank PSUM tile so a single exp call
                # covers them); Z/PV trail the score/exp stream by one
                # chunk, software-pipelined ACROSS q-groups so the PE never
                # drains at a group boundary. The 1/Z broadcast borrows po
                # slots.
                with tc.tile_pool(name="st_ps", bufs=4, space="PSUM") as stp, \
                     tc.tile_pool(name="pv_ps", bufs=1, space="PSUM") as pvp, \
                     tc.tile_pool(name="z_ps", bufs=1, space="PSUM") as zpp:
                    norm_q = []  # pending (qg, h) normalization steps

                    def norm_step():
                        nqg, h = norm_q.pop(0)
                        qs0 = nqg * TB
                        zbp = pvp.tile(
                            [128, TB], F32, tag=f"po{h}", name="zbp"
                        )
                        nc.tensor.matmul(
                            zbp,
                            lhsT=ones_row,
                            rhs=zrs_tab[0:1, h * T + qs0 : h * T + qs0 + TB],
                            start=True,
                            stop=True,
                        )
                        nc.vector.tensor_mul(
                            oT[:, h, qs0 : qs0 + TB],
                            oT[:, h, qs0 : qs0 + TB],
                            zbp,
                        )
                    # (qg, j) chunk schedule, flattened; (0,0) was
                    # pre-warmed inside the projection scope
                    sched = [
                        (qg, j)
                        for qg in range(NTB_B)
                        for j in range(JPG * (qg + 1))
                    ][2:]
                    po = {}
                    zrow = {}
                    pend = warm  # [qg, j, cur, z_done]

                    def z_step(pqg, pj, pts):
                        """Z matmuls (trail 2); zrow allocates lazily at the
                        group's first step; recips fire on the last chunk so
                        the z slots recycle early."""
                        pjmax = JPG * (pqg + 1)
                        if (pqg, 0) not in zrow:
                            for h in range(HPC):
                                zrow[pqg, h] = zpp.tile(
                                    [1, TB], F32, tag=f"z{h}", name=f"z{h}"
                                )
                        for h in range(HPC):
                            ptp, offp = pts[h]
                            nc.tensor.matmul(
                                zrow[pqg, h][:, offp:],
                                lhsT=ones_col,
                                rhs=ptp[:, offp:],
                                start=(pj == 0),
                                stop=(pj == pjmax - 1),
                            )
                        if pj == pjmax - 1:
                            qs0 = pqg * TB
                            for h in range(HPC):
                                with nc.allow_low_precision(
                                    reason="bf16 1/Z: 0.4% rel, in tolerance"
                                ):
                                    nc.vector.reciprocal(
                                        zrs_tab[
                                            0:1, h * T + qs0 : h * T + qs0 + TB
                                        ],
                                        zrow[pqg, h],
                                    )

                    def pv_step(pqg, pj, pts):
                        """PV matmuls (trail 3); po allocates lazily after
                        pending 1/Z broadcasts take the po-tag slots; po
                        drains on the last chunk."""
                        pjmax = JPG * (pqg + 1)
                        if (pqg, 0) not in po:
                            while norm_q:
                                norm_step()
                            for h in range(HPC):
                                po[pqg, h] = pvp.tile(
                                    [128, TB], F32, tag=f"po{h}", name=f"po{h}"
                                )
                        for h in range(HPC):
                            ptp, offp = pts[h]
                            nc.tensor.matmul(
                                po[pqg, h][:, offp:],
                                lhsT=vN[:, pj, h * HD : (h + 1) * HD],
                                rhs=ptp[:, offp:],
                                start=(pj == 0),
                                stop=(pj == pjmax - 1),
                            )
                        if pj == pjmax - 1:
                            qs0 = pqg * TB
                            last_g = pqg == NTB_B - 1
                            for h in range(HPC):
                                if last_g:
                                    nc.scalar.activation(
                                        oT[:, h, qs0 : qs0 + TB],
                                        po[pqg, h],
                                        Copy,
                                    )
                                else:
                                    nc.vector.tensor_copy(
                                        oT[:, h, qs0 : qs0 + TB], po[pqg, h]
                                    )
                                norm_q.append((pqg, h))

                    for qg, j in sched:
                        qs0 = qg * TB
                        off = max(0, (j - JPG * qg) * 128)
                        ks = slice(j * 128, (j + 1) * 128)
                        cur = {}
                        for h in range(HPC):
                            st = stp.tile([128, TB], F32, tag="st")
                            nc.tensor.matmul(
                                st[:, off:],
                                lhsT=kT[:, h, ks],
                                rhs=qT[:, h, qs0 + off : qs0 + TB],
                                start=True,
                                stop=True,
                            )
                            if j >= JPG * qg:  # diagonal 128-block mask
                                nc.vector.tensor_add(
                                    st[:, off : off + 128],
                                    st[:, off : off + 128],
                                    negm,
                                )
                            pt = asb.tile([128, TB], BF16, tag="pt")
                            nc.scalar.activation(
                                pt[:, off:], st[:, off:], Exp, scale=SCALE
                            )
                            cur[h] = (pt, off)
                        pend.append([qg, j, cur, False])
                        if len(pend) >= 3:
                            it = pend[-3]
                            z_step(it[0], it[1], it[2])
                            it[3] = True
                        if len(pend) >= 4:
                            it = pend.pop(0)
                            pv_step(it[0], it[1], it[2])
                    for it in pend:
                        if not it[3]:
                            z_step(it[0], it[1], it[2])
                    while pend:
                        it = pend.pop(0)
                        pv_step(it[0], it[1], it[2])
                    while norm_q:
                        norm_step()

                _mark(nc, f"b{b}_outproj")
                # ---------------- out-projection (partial) ----------------
                # output stores grouped 4 column-blocks per DMA (SP-seq time
                # per DMA is the store-path bottleneck, not bandwidth)
                with tc.tile_pool(name="fo_ps", bufs=1, space="PSUM") as fop:
                    for tb in range(NTB_B):
                        tbs = slice(tb * TB, (tb + 1) * TB)
                        fs = None
                        for nb in range(D // 128):
                            nbs = slice(nb * 128, (nb + 1) * 128)
                            fo = fop.tile(
                                [128, TB], F32, tag=f"fo{nb % 4}",
                                name=f"fo{nb % 4}",
                            )
                            for m in range(HPC):
                                nc.tensor.matmul(
                                    fo,
                                    lhsT=wo_sb[:, m, nbs],
                                    rhs=oT[:, m, tbs],
                                    start=(m == 0),
                                    stop=(m == HPC - 1),
                                )
                            grp = (
                                2
                                if (b == B - 1 and tb == NTB_B - 1 and nb >= 12)
                                else 4
                            )
                            if nb % grp == 0:
                                fs = fsb.tile(
                                    [128, 4, TB], F16, tag="fs"
                                )
                            if nb % 2 == 1:
                                nc.vector.tensor_copy(fs[:, nb % grp, :], fo)
                            else:
                                nc.scalar.activation(
                                    fs[:, nb % grp, :], fo, Copy
                                )
                            if nb % grp == grp - 1:
                                last = (
                                    b == B - 1
                                    and tb == NTB_B - 1
                                    and nb == D // 128 - 1
                                )
                                deng = nc.scalar if last else nc.sync
                                deng.dma_start(
                                    out=out_v[
                                        :,
                                        nb - grp + 1 : nb + 1,
                                        t0 + tb * TB : t0 + (tb + 1) * TB,
                                    ],
                                    in_=fs[:, 0:grp, :],
                                )
    _legalize_waits(nc)
    return nc


_NC_CACHE = None


def _get_program():
    global _NC_CACHE
    if _NC_CACHE is None:
        _NC_CACHE = build_program()
    return _NC_CACHE


def _rope_tables():
    inv_freq = 1.0 / (ROPE_THETA ** (np.arange(0, HD, 2, dtype=np.float32) / HD))
    freqs = np.arange(T, dtype=np.float32)[:, None] * inv_freq[None, :]  # (T, 64)
    emb = np.concatenate([freqs, freqs], axis=-1)                        # (T, 128)
    cosT = np.ascontiguousarray(np.cos(emb).T).astype(BF16_NP)           # [128, T]
    sinT = np.sin(emb).T.astype(np.float32)
    sinhT = np.ascontiguousarray(
        np.concatenate([-sinT[: HD // 2], sinT[HD // 2 :]], axis=0)
    ).astype(BF16_NP)
    return cosT, sinhT


def kernel(x, Wq, Wk, Wv, Wo, **run_kwargs):
    x = np.asarray(x, dtype=np.float32)
    Wq = np.asarray(Wq, dtype=np.float32)
    Wk = np.asarray(Wk, dtype=np.float32)
    Wv = np.asarray(Wv, dtype=np.float32)
    Wo = np.asarray(Wo, dtype=np.float32)

    nc = _get_program()
    cosT, sinhT = _rope_tables()
    xT = np.ascontiguousarray(x.reshape(BT, D).T).astype(BF16_NP)  # [D, BT]
    # S^T[tk, tq] causal mask for the diagonal block: keep tq(col) >= tk(row)
    r = np.arange(128)
    negmM = np.where(r[None, :] >= r[:, None], 0.0, -1e30).astype(np.float32)

    in_maps = []
    for c in range(NCORES):
        sl = slice(c * M_PC, (c + 1) * M_PC)
        in_maps.append(
            {
                "xT": xT,
                "negmM": negmM,
                "wqkT": np.ascontiguousarray(
                    np.concatenate([Wq[sl, :].T, Wk[sl, :].T], axis=1)
                ).astype(BF16_NP),
                "wvT": np.ascontiguousarray(Wv[sl, :].T).astype(BF16_NP),
                "woT": np.ascontiguousarray(Wo[:, sl].T).astype(BF16_NP),
                "cosT": cosT,
                "sinhT": sinhT,
            }
        )

    res = run_bass_kernel_spmd(nc, in_maps, list(range(NCORES)), **run_kwargs)
    acc = np.zeros((D, BT), dtype=np.float32)
    for c in range(NCORES):
        acc += res.results[c]["partialT"].astype(np.float32)
    out = np.ascontiguousarray(acc.T).reshape(B, T, D)
    if run_kwargs:
        return out, res
    return out



# revision 45
# speedup vs baseline: 1.0704x; 1.0704x over previous
"""Multi-head self-attention (B=2, T=2048, D=2048, H=16, RoPE, causal)
as a Bass/Tile kernel running SPMD on 8 trn2 NeuronCores.

Sharding: tensor-parallel over heads (2 heads per core). Each core
computes its heads' Q/K/V projections, RoPE, causal attention, and a
partial out-projection over its 256 feature columns; the host sums the
8 partial outputs (all-reduce equivalent).

Dataflow (per core, per batch):
  - x streamed per 512-wide t-block ([128, 16, 512] SBUF tiles, 4 tags);
    the first block's DMA is interleaved per-contraction-chunk with the
    weight loads so the PE starts ~2us in.
  - Q/K projections in "T-layout" (feature dim on partitions, time on
    free); RoPE rotate-half via two SBUF->SBUF partition-swap DMAs
    (sign folded into the sin table), all-bf16 combine on DVE (2x mode).
  - V projected directly in natural layout ([tk, d]): lhsT = x chunk,
    rhs = Wv slice -- no PE transposes.
  - scores computed transposed: S^T[tk, tq] per (key-chunk, q-group).
    Chunks are narrowed to the causal region (exact 136-block lower
    triangle, no fully-masked work); only the diagonal 128x128 block
    gets a mask add. The two heads' chunk streams are interleaved so
    the PE always has ~1.3us of work while exp round-trips through
    DVE/Act. Z column sums via transposed matmuls (lhsT = exp chunk,
    rhs = ones column, [128,1] output) accumulated in PSUM -- the PE
    streams 1 column per chunk instead of up-to-512.
  - normalization trails each q-group: po -> oT (unnormalized cast);
    Z columns -> DVE reciprocal -> PE transpose -> Act copy -> an
    8-descriptor DMA back into row layout; then a ones-row broadcast
    matmul (riding the score-tile PSUM ring) + in-place DVE multiply,
    paced several chunks behind so the DMA latency stays off the PE.
  - out-projection accumulates the two head-chunks in PSUM; partial
    result cast to f16 and DMA'd out; host sums partials across cores.
"""

import sys

sys.path.insert(0, "/opt/trn_rl_repo")

import ml_dtypes
import numpy as np

import concourse.bass as bass
import concourse.mybir as mybir
import concourse.tile as tile
from concourse.bass_utils import run_bass_kernel_spmd


def _legalize_waits(nc):
    """Walrus codegen rejects >2 sync waits on DMA/matmul/nop-class
    instructions, and Tile's pool-recycle waits bypass its own elision.
    Spill excess waits (>1) onto freshly inserted same-engine NoOps
    placed immediately before the offending instruction (sound w.r.t.
    per-engine program order)."""
    spill_id = [0]
    for bb in nc.m.functions[0].blocks:
        new_insts = []
        for inst in bb.instructions:
            si = getattr(inst, "sync_info", None)
            if si is None or not si.on_wait:
                new_insts.append(inst)
                continue
            eng = getattr(inst, "engine", None)
            kept = list(si.on_wait)
            if len(kept) > 1 and eng is not None:
                excess, kept = kept[:-1], kept[-1:]
                for w in excess:
                    spill_id[0] += 1
                    nop = mybir.InstNoOp(
                        name=f"I-wspill-{spill_id[0]}",
                        ins=[],
                        outs=[],
                        engine=eng,
                    )
                    nop.sync_info = mybir.SyncInfo(on_wait=[w], on_update=[])
                    new_insts.append(nop)
            if len(kept) != len(si.on_wait):
                si.on_wait[:] = kept
            new_insts.append(inst)
        if len(new_insts) != len(bb.instructions):
            bb.instructions[:] = new_insts


_PHASE_MARKS = []  # (phase_label, last_inst_index_before_phase) - profiling aid


def _mark(nc, label):
    n = -1
    for fn in nc.m.functions:
        for bb in fn.blocks:
            for ins in bb.instructions:
                if ins.name.startswith("I-"):
                    try:
                        n = max(n, int(ins.name[2:]))
                    except ValueError:
                        pass
    _PHASE_MARKS.append((label, n))


B, T, D, H, HD = 2, 2048, 2048, 16, 128
NCORES = 8
HPC = H // NCORES            # heads per core = 2
M_PC = HPC * HD              # per-core feature slice = 256
BT = B * T                   # 4096
SCALE = HD ** -0.5
ROPE_THETA = 10000.0

F32 = mybir.dt.float32
F16 = mybir.dt.float16
BF16 = mybir.dt.bfloat16
BF16_NP = ml_dtypes.bfloat16

TB = 512                     # t-block for projections / q-groups
NTB_B = T // TB              # 4 t-blocks per batch
NMC = D // 128               # 16 contraction chunks
NKC = T // 128               # 16 key chunks per batch
JPG = TB // 128              # key chunks per q-group width = 4

Copy = mybir.ActivationFunctionType.Copy
Exp = mybir.ActivationFunctionType.Exp


def build_program():
    nc = bass.Bass()

    xT_d = nc.declare_dram_parameter("xT", [D, BT], BF16, isOutput=False)
    tri_d = nc.declare_dram_parameter("triT", [128, 128], BF16, isOutput=False)
    # wq and wk concatenated so one DMA covers both (halves SP-seq time
    # on the critical startup path)
    wqk_d = nc.declare_dram_parameter(
        "wqkT", [D, 2 * M_PC], BF16, isOutput=False
    )
    wv_d = nc.declare_dram_parameter("wvT", [D, M_PC], BF16, isOutput=False)
    wo_d = nc.declare_dram_parameter("woT", [M_PC, D], BF16, isOutput=False)
    ident_d = nc.declare_dram_parameter("identT", [128, 128], BF16, isOutput=False)
    cos_d = nc.declare_dram_parameter("cosT", [HD, T], BF16, isOutput=False)
    sinh_d = nc.declare_dram_parameter("sinhT", [HD, T], BF16, isOutput=False)
    out_d = nc.declare_dram_parameter("partialT", [D, BT], F16, isOutput=True)

    xT_v = xT_d.rearrange("(c p) t -> p c t", p=128)      # [128, 16, BT]
    wqk_v = wqk_d.rearrange("(c p) n -> p c n", p=128)    # [128, 16, 512]
    wv_v = wv_d.rearrange("(c p) n -> p c n", p=128)
    wo_v = wo_d.rearrange("(c p) n -> p c n", p=128)      # [128, 2, 2048]
    out_v = out_d.rearrange("(c p) t -> p c t", p=128)    # [128, 16, BT]

    with tile.TileContext(nc) as tc:
        with (
            tc.tile_pool(name="wpool", bufs=1) as wpool,
            tc.tile_pool(name="xp", bufs=1) as xp,
            tc.tile_pool(name="big", bufs=1) as big,
            tc.tile_pool(name="rp", bufs=2) as rp,
            tc.tile_pool(name="attn_sb", bufs=8) as asb,
            tc.tile_pool(name="fs_sb", bufs=6) as fsb,
        ):
            # ---- weights + first x block, interleaved in graduated mc
            # groups (fast pipeline fill, then few big SP-cheap DMAs) ----
            wqk_sb = wpool.tile([128, NMC, 2 * M_PC], BF16, tag="wqk")
            wv_sb = wpool.tile([128, NMC, M_PC], BF16, tag="wv")
            x_tiles = {}
            xt0 = xp.tile([128, NMC, TB], BF16, tag="x0", name="x_b0_t0")
            x_tiles[(0, 0)] = xt0
            for lo, hi in ((0, 1), (1, 2), (2, 3), (3, 4), (4, 6), (6, 8),
                           (8, 10), (10, 12), (12, 14), (14, 16)):
                nc.sync.dma_start(
                    out=wqk_sb[:, lo:hi, :], in_=wqk_v[:, lo:hi, :]
                )
                # first x chunk rides the idle DVE queue, in parallel with
                # SP's weight DMA, to cut the cold-start latency
                eng = nc.scalar if lo == 0 else nc.sync
                eng.dma_start(
                    out=xt0[:, lo:hi, :], in_=xT_v[:, lo:hi, 0:TB]
                )

            cos_sb = wpool.tile([128, T], BF16, tag="cos")
            sinh_sb = wpool.tile([128, T], BF16, tag="sinh")
            nc.sync.dma_start(out=cos_sb[:, 0:TB], in_=cos_d[:, 0:TB])
            nc.sync.dma_start(out=sinh_sb[:, 0:TB], in_=sinh_d[:, 0:TB])

            def load_x(b, tb):
                t = xp.tile(
                    [128, NMC, TB], BF16, tag=f"x{tb}", name=f"x_b{b}_t{tb}"
                )
                x_tiles[(b, tb)] = t
                lo = b * T + tb * TB
                for m0 in range(0, NMC, 4):
                    nc.sync.dma_start(
                        out=t[:, m0 : m0 + 4, :],
                        in_=xT_v[:, m0 : m0 + 4, lo : lo + TB],
                    )

            # wv rides alongside tb0's V matmuls; x block 1 follows
            for m0 in range(0, NMC, 4):
                nc.sync.dma_start(
                    out=wv_sb[:, m0 : m0 + 4, :], in_=wv_v[:, m0 : m0 + 4, :]
                )
            load_x(0, 1)
            nc.sync.dma_start(out=cos_sb[:, TB:], in_=cos_d[:, TB:])
            nc.sync.dma_start(out=sinh_sb[:, TB:], in_=sinh_d[:, TB:])
            tri01 = wpool.tile([128, 128], BF16, tag="tri01")
            nc.sync.dma_start(out=tri01, in_=tri_d[:, :])
            ident = wpool.tile([128, 128], BF16, tag="ident")
            nc.sync.dma_start(out=ident, in_=ident_d[:, :])
            ones_col = wpool.tile([128, 1], BF16, tag="ones_c")
            nc.vector.memset(ones_col, 1.0)
            ones_row = wpool.tile([1, 128], BF16, tag="ones_r")
            nc.vector.memset(ones_row, 1.0)

            for tb in range(2, NTB_B):
                load_x(0, tb)

            wo_sb = wpool.tile([128, HPC, D], BF16, tag="wo")
            nc.sync.dma_start(out=wo_sb, in_=wo_v)

            for b in range(B):
                t0 = b * T  # global t offset of this batch
                _mark(nc, f"b{b}_proj")

                # persistent per-batch tensors (slots reused across b)
                qT = big.tile([128, HPC, T], BF16, tag="qT")   # [hd, h, t]
                kT = big.tile([128, HPC, T], BF16, tag="kT")
                vN = big.tile([128, NKC, M_PC], BF16, tag="vN")  # [tk, j, n]
                oT = big.tile([128, HPC, T], BF16, tag="oT")   # attn out

                # ---------------- projections + RoPE ----------------
                warm = []  # pre-emitted attention chunks
                with tc.tile_pool(name="qk_ps", bufs=1, space="PSUM") as qkp, \
                     tc.tile_pool(name="v_ps", bufs=1, space="PSUM") as vps:
                    for tb in range(NTB_B):
                        xt = x_tiles[(b, tb)]
                        ts_l = slice(tb * TB, (tb + 1) * TB)   # in-batch
                        ps = {}
                        for h in range(HPC):
                            for nm in ("q", "k"):
                                ps[nm, h] = qkp.tile(
                                    [128, TB], F32, tag=f"{nm}{h}",
                                    name=f"ps_{nm}{h}",
                                )
                        for mc in range(NMC):
                            for h in range(HPC):
                                for ni, nm in ((0, "q"), (1, "k")):
                                    hs = slice(
                                        ni * M_PC + h * HD,
                                        ni * M_PC + (h + 1) * HD,
                                    )
                                    nc.tensor.matmul(
                                        ps[nm, h],
                                        lhsT=wqk_sb[:, mc, hs],
                                        rhs=xt[:, mc, :],
                                        start=(mc == 0),
                                        stop=(mc == NMC - 1),
                                    )
                        # raw bf16 casts early on Act (deps ready now)
                        raws = {}
                        for nm in ("q", "k"):
                            for h in range(HPC):
                                raw = rp.tile(
                                    [128, TB], BF16, tag=f"raw{nm}{h}"
                                )
                                nc.scalar.activation(raw, ps[nm, h], Copy)
                                raws[nm, h] = raw
                        # RoPE before V: the qk/sw PSUM banks drain while the
                        # PE streams V matmuls, so the next phase's bank-WAR
                        # waits resolve before the PE gets there
                        for nm, dest in (("q", qT), ("k", kT)):
                            for h in range(HPC):
                                raw = raws[nm, h]
                                swb = rp.tile([128, TB], BF16, tag="swb")
                                nc.sync.dma_start(
                                    out=swb[0:64, :], in_=raw[64:128, :]
                                )
                                nc.sync.dma_start(
                                    out=swb[64:128, :], in_=raw[0:64, :]
                                )
                                t2 = rp.tile([128, TB], BF16, tag="t2")
                                nc.vector.tensor_mul(t2, swb, sinh_sb[:, ts_l])
                                t1 = rp.tile([128, TB], BF16, tag="t1")
                                nc.vector.tensor_mul(t1, raw, cos_sb[:, ts_l])
                                nc.vector.tensor_add(dest[:, h, ts_l], t1, t2)
                        # V directly in natural layout: per 128-wide tk chunk
                        def v_chunk(s):
                            j = tb * JPG + s
                            pv = vps.tile(
                                [128, M_PC], F32, tag=f"v{s % 2}",
                                name=f"ps_v{s % 2}",
                            )
                            for mc in range(NMC):
                                nc.tensor.matmul(
                                    pv,
                                    lhsT=xt[:, mc, s * 128 : (s + 1) * 128],
                                    rhs=wv_sb[:, mc, :],
                                    start=(mc == 0),
                                    stop=(mc == NMC - 1),
                                )
                            nc.scalar.activation(vN[:, j, :], pv, Copy)

                        for s in range(JPG):
                            # tb2/tb3's V chunks are deferred into the
                            # attention chunks (the window is Act-bound
                            # there; these give the PE ~1.7us each and move
                            # their Act/DVE load out of the backlog)
                            if tb < NTB_B - 2:
                                v_chunk(s)
                            # between tb3's V chunks, warm up attention: the
                            # (qg0, j0) score pair + exp run in drained qk
                            # PSUM slots, their latency hidden by V matmuls,
                            # so the attention loop starts with a full
                            # pipeline instead of a fill bubble
                            if tb == NTB_B - 1 and s in (1, 2):
                                # chunk (0,0) after V s1 in the q-tag slots,
                                # chunk (0,1) after V s2 in the k-tag slots:
                                # the attention loop starts two chunks deep,
                                # matching the Z trail exactly
                                wj = 0 if s == 1 else 1
                                woff = 0 if wj == 0 else 128
                                wtag = "q" if s == 1 else "k"
                                wpt2 = asb.tile(
                                    [128, HPC, TB], BF16, tag="pt2",
                                    name="warm_pt",
                                )
                                for h in range(HPC):
                                    wst = qkp.tile(
                                        [128, TB], F32,
                                        tag=f"{wtag}{h}", name="warm_st",
                                    )
                                    nc.tensor.matmul(
                                        wst[:, woff:],
                                        lhsT=kT[:, h, wj * 128 : (wj + 1) * 128],
                                        rhs=qT[:, h, woff:TB],
                                        start=True,
                                        stop=True,
                                    )
                                    nc.scalar.activation(
                                        wpt2[:, h, woff:], wst[:, woff:],
                                        Exp, scale=SCALE,
                                    )
                                    nc.vector.tensor_mul(
                                        wpt2[:, h, woff : woff + 128],
                                        wpt2[:, h, woff : woff + 128],
                                        tri01,
                                    )
                                warm.append([0, wj, (wpt2, woff), False])



                _mark(nc, f"b{b}_attn")
                # ---------------- attention + out-projection ----------------
                # Both heads' score tiles share one 2-bank PSUM tile so a
                # single exp call covers them (Act is the scarce engine in
                # this window). Z/PV trail the score/exp stream by 2/3
                # chunks, software-pipelined ACROSS q-groups. Z is
                # accumulated as [128,1] columns via transposed matmuls
                # (exp chunk stationary, ones moving) -- ~1 PE row per chunk
                # instead of up-to-512 -- and routed back to row layout off
                # the PE (recip -> PE transpose -> Act copy -> 8-descriptor
                # DMA). The out-projection interleaves with the chunk
                # stream, one 128-column block per chunk once its q-group is
                # normalized, keeping the PE fed while Act drains exps; the
                # leftovers plus the last group's norms form the batch tail
                # on the score-tile ring.
                with tc.tile_pool(name="st_ps", bufs=2, space="PSUM") as stp, \
                     tc.tile_pool(name="pv_ps", bufs=1, space="PSUM") as pvp, \
                     tc.tile_pool(name="z_ps", bufs=1, space="PSUM") as zpp, \
                     tc.tile_pool(name="fo_ps", bufs=1, space="PSUM") as fop:
                    norm_q = []  # pending [qg, h, ready_at_pv_call]
                    zr_rows = {}  # qg -> [1, HPC*JPG, 128] recip-Z row tile
                    zr_pend = []  # [qg, zrcol, z_call_at_emit]
                    ob_q = []  # (tb, nb) out-proj blocks ready to emit
                    fs_state = {}  # tb -> current fs store-group tile
                    n_z = [0]
                    n_pv = [0]

                    def norm_step(pool, tag):
                        nqg, h, _ = norm_q.pop(0)
                        qs0 = nqg * TB
                        zbp = pool.tile([128, TB], F32, tag=tag, name="zbp")
                        nc.tensor.matmul(
                            zbp,
                            lhsT=ones_row,
                            rhs=zr_rows[nqg][0:1, h * JPG : (h + 1) * JPG, :],
                            start=True,
                            stop=True,
                        )
                        nc.vector.tensor_mul(
                            oT[:, h, qs0 : qs0 + TB],
                            oT[:, h, qs0 : qs0 + TB],
                            zbp,
                        )
                        if h == HPC - 1:  # group fully normalized
                            ob_q.extend(
                                (nqg, nb) for nb in range(D // 128)
                            )

                    def emit_block(btb, nb, pool, tag):
                        """One 128-column out-projection block: 2 matmuls
                        into a borrowed PSUM slot, cast into the current
                        4-block store group, DMA when the group completes."""
                        tbs = slice(btb * TB, (btb + 1) * TB)
                        nbs = slice(nb * 128, (nb + 1) * 128)
                        fo = pool.tile([128, TB], F32, tag=tag, name="fo")
                        for m in range(HPC):
                            nc.tensor.matmul(
                                fo,
                                lhsT=wo_sb[:, m, nbs],
                                rhs=oT[:, m, tbs],
                                start=(m == 0),
                                stop=(m == HPC - 1),
                            )
                        if b == B - 1 and btb == NTB_B - 1 and nb >= 12:
                            # shrink the final stores: the last DMA's
                            # latency is the kernel's tail
                            grp = 2 if nb < 14 else 1
                        else:
                            grp = 4
                        if nb % grp == 0:
                            fs_state[btb] = fsb.tile(
                                [128, 4, TB], F16, tag="fs", name="fs"
                            )
                        fs = fs_state[btb]
                        # 9/16 of casts on Act (DVE also carries the norm
                        # muls and zr copies); the kernel's final two casts
                        # go to Act, whose queue is empty at the end
                        on_act = nb % 2 == 0 or nb % 16 == 7 or grp == 1
                        if on_act:
                            nc.scalar.activation(fs[:, nb % grp, :], fo, Copy)
                        else:
                            nc.vector.tensor_copy(fs[:, nb % grp, :], fo)
                        if nb % grp == grp - 1:
                            # stores ride the otherwise-idle Pool queue (SP
                            # serializes ~1.6us per store; x loads need it);
                            # the kernel's very last store goes to the Act
                            # queue, which is empty at the end
                            last = (
                                b == B - 1
                                and btb == NTB_B - 1
                                and nb == D // 128 - 1
                            )
                            deng = nc.scalar if last else nc.gpsimd
                            deng.dma_start(
                                out=out_v[
                                    :,
                                    nb - grp + 1 : nb + 1,
                                    t0 + btb * TB : t0 + (btb + 1) * TB,
                                ],
                                in_=fs[:, 0:grp, :],
                            )

                    def process_zr():
                        """Late half of the Z path: PE-transpose the recip-Z
                        columns to [8,128] (riding the out-proj PSUM slot),
                        copy PSUM->SBUF on Act, and DMA the 8 rows into a
                        single [1, 8*128] row for the broadcast matmul."""
                        zqg, zrcol, _ = zr_pend.pop(0)
                        zrT = fop.tile(
                            [HPC * JPG, 128], BF16, tag="foA", name="zrT"
                        )
                        nc.tensor.transpose(zrT, zrcol, ident)
                        zrS = rp.tile([HPC * JPG, 128], BF16, tag="zs8")
                        # DVE, not Act: at the batch tail Act still has the
                        # trailing exps queued, which would delay the DMA
                        nc.vector.tensor_copy(zrS, zrT)
                        zrow_t = rp.tile(
                            [1, HPC * JPG, 128], BF16, tag="zrow"
                        )
                        # idle Pool queue: the SP queue carries x loads and
                        # output stores, which would delay this tiny DMA
                        nc.gpsimd.dma_start(out=zrow_t, in_=zrS)
                        zr_rows[zqg] = zrow_t
                        for h in range(HPC):
                            norm_q.append([zqg, h, n_pv[0] + 5 + 2 * h])

                    # (qg, j) chunk schedule, flattened; (0,0) was
                    # pre-warmed inside the projection scope
                    sched = [
                        (qg, j)
                        for qg in range(NTB_B)
                        for j in range(JPG * (qg + 1))
                    ][2:]
                    po = {}
                    zcol = {}
                    pend = warm  # [qg, j, (pt2, off), z_done]

                    def z_step(pqg, pj, pts):
                        """Z partial sums (trail 2) as transposed matmuls:
                        one [128,1] PSUM column per (head, 128-wide tq sub),
                        accumulated over key chunks. Each column's stop fires
                        on its last causal chunk; the recip covers all 8
                        columns in one DVE op on the group's last chunk."""
                        n_z[0] += 1
                        if zr_pend and n_z[0] - zr_pend[0][2] >= 1:
                            process_zr()
                        base = JPG * pqg
                        if pqg not in zcol:
                            zcol[pqg] = zpp.tile(
                                [128, HPC * JPG], F32, tag="zc", name="zc"
                            )
                        zc = zcol[pqg]
                        pt2, offp = pts
                        sub0 = max(0, pj - base)
                        # all 8 columns share ONE accumulation group (PSUM
                        # groups are tracked per 2KB zero region = the whole
                        # bank): start only on the group's first matmul --
                        # its start bit marks the bank pending-zero, so the
                        # other columns' first writes see zeros -- and stop
                        # only on its very last
                        for h in range(HPC):
                            for s in range(sub0, JPG):
                                nc.tensor.matmul(
                                    zc[:, h * JPG + s : h * JPG + s + 1],
                                    lhsT=pt2[:, h, s * 128 : (s + 1) * 128],
                                    rhs=ones_col,
                                    start=(pj == 0 and h == 0 and s == 0),
                                    stop=(
                                        pj == JPG * (pqg + 1) - 1
                                        and h == HPC - 1
                                        and s == JPG - 1
                                    ),
                                )
                        if pj == JPG * (pqg + 1) - 1:
                            zrcol = rp.tile(
                                [128, HPC * JPG], BF16, tag="zrc"
                            )
                            with nc.allow_low_precision(
                                reason="bf16 1/Z: 0.4% rel, in tolerance"
                            ):
                                nc.vector.reciprocal(zrcol, zc)
                            zr_pend.append([pqg, zrcol, n_z[0]])

                    def pv_step(pqg, pj, pts):
                        """PV matmuls (trail 3); one paced 1/Z broadcast may
                        ride along per call once its DMA has had >= 5 chunks
                        to land. Returns True if a broadcast was emitted."""
                        n_pv[0] += 1
                        popped = False
                        if norm_q and norm_q[0][2] <= n_pv[0]:
                            norm_step(fop, "foA")
                            popped = True
                        pjmax = JPG * (pqg + 1)
                        if (pqg, 0) not in po:
                            for h in range(HPC):
                                po[pqg, h] = pvp.tile(
                                    [128, TB], F32, tag=f"po{h}", name=f"po{h}"
                                )
                        pt2, offp = pts
                        for h in range(HPC):
                            nc.tensor.matmul(
                                po[pqg, h][:, offp:],
                                lhsT=vN[:, pj, h * HD : (h + 1) * HD],
                                rhs=pt2[:, h, offp:],
                                start=(pj == 0),
                                stop=(pj == pjmax - 1),
                            )
                        if pj == pjmax - 1:
                            qs0 = pqg * TB
                            for h in range(HPC):
                                nc.vector.tensor_copy(
                                    oT[:, h, qs0 : qs0 + TB], po[pqg, h]
                                )
                        return popped

                    # deferred V chunks: tb2's early (its vN rows are needed
                    # by group 2's PV), tb3's late; each takes the first
                    # conflict-free foA slot at/after its target chunk
                    defer_v = [(NTB_B - 2, s, (0, 1, 3, 5)[s]) for s in range(JPG)]
                    defer_v += [(NTB_B - 1, s, (21, 23, 25, 27)[s]) for s in range(JPG)]

                    def emit_v(vtb, s):
                        jv = vtb * JPG + s
                        xt_v_src = x_tiles[(b, vtb)]
                        pv = fop.tile(
                            [128, M_PC], F32, tag="foA", name="ps_vd"
                        )
                        for mc in range(NMC):
                            nc.tensor.matmul(
                                pv,
                                lhsT=xt_v_src[:, mc, s * 128 : (s + 1) * 128],
                                rhs=wv_sb[:, mc, :],
                                start=(mc == 0),
                                stop=(mc == NMC - 1),
                            )
                        nc.vector.tensor_copy(vN[:, jv, :], pv)

                    for n_ch, (qg, j) in enumerate(sched):
                        qs0 = qg * TB
                        off = max(0, (j - JPG * qg) * 128)
                        ks = slice(j * 128, (j + 1) * 128)
                        st2 = stp.tile([128, HPC, TB], F32, tag="st2")
                        for h in range(HPC):
                            nc.tensor.matmul(
                                st2[:, h, off:],
                                lhsT=kT[:, h, ks],
                                rhs=qT[:, h, qs0 + off : qs0 + TB],
                                start=True,
                                stop=True,
                            )
                        # exp first (it's on the PE's st2-ring critical
                        # path), then zero the diagonal block's masked wedge
                        # on the SBUF side where the 2-chunk Z/PV trail
                        # hides the DVE latency
                        pt2 = asb.tile([128, HPC, TB], BF16, tag="pt2")
                        nc.scalar.activation(
                            pt2[:, :, off:], st2[:, :, off:], Exp, scale=SCALE
                        )
                        if j >= JPG * qg:
                            for h in range(HPC):
                                nc.vector.tensor_mul(
                                    pt2[:, h, off : off + 128],
                                    pt2[:, h, off : off + 128],
                                    tri01,
                                )
                        pend.append([qg, j, (pt2, off), False])
                        popped = False
                        if len(pend) >= 4:
                            it = pend.pop(0)
                            popped = pv_step(it[0], it[1], it[2])
                        if (
                            defer_v
                            and n_ch >= defer_v[0][2]
                            and not popped
                            and not zr_pend
                        ):
                            vtb, vs, _ = defer_v.pop(0)
                            emit_v(vtb, vs)
                        if b == 0 and n_ch == 10:
                            # b1's x loads: emitted once b0's deferred V
                            # matmuls (the last xt readers) are in the stream
                            for ltb in range(NTB_B):
                                load_x(1, ltb)
                        if len(pend) >= 3:
                            it = pend[-3]
                            z_step(it[0], it[1], it[2])
                            it[3] = True
                    for it in pend:
                        if not it[3]:
                            z_step(it[0], it[1], it[2])
                    while pend:
                        it = pend.pop(0)
                        pv_step(it[0], it[1], it[2])
                    # batch tail: the last group's Z path (transpose after
                    # the PV flush so the recip has landed), then leftover
                    # out-proj blocks interleaved with the remaining 1/Z
                    # broadcasts on the 2-bank score ring -- blocks ahead of
                    # each norm cover the zr DMA latency.
                    while zr_pend:
                        process_zr()
                    _mark(nc, f"b{b}_outproj")
                    # tail: all remaining out-proj blocks, rotating over
                    # three idle PSUM slots (half an st2 chunk slot + the
                    # two drained po banks) so each block's WAR lands on a
                    # cast three blocks back; the last group's norms pop
                    # between batches of four, after enough blocks have
                    # covered the zr DMA latency.
                    rot = [
                        (stp, "st2"),
                        (pvp, "po0"),
                        (pvp, "po1"),
                        (fop, "foA"),
                    ]
                    r = [0]

                    def tail_slot():
                        p, tg = rot[r[0] % 4]
                        r[0] += 1
                        return p, tg

                    first = True
                    while ob_q or norm_q:
                        k = 0
                        while ob_q and k < (8 if first else 4):
                            emit_block(*ob_q.pop(0), *tail_slot())
                            k += 1
                        first = False
                        if norm_q:
                            norm_step(*tail_slot())
    _legalize_waits(nc)
    return nc


_NC_CACHE = None


def _get_program():
    global _NC_CACHE
    if _NC_CACHE is None:
        _NC_CACHE = build_program()
    return _NC_CACHE


def _rope_tables():
    inv_freq = 1.0 / (ROPE_THETA ** (np.arange(0, HD, 2, dtype=np.float32) / HD))
    freqs = np.arange(T, dtype=np.float32)[:, None] * inv_freq[None, :]  # (T, 64)
    emb = np.concatenate([freqs, freqs], axis=-1)                        # (T, 128)
    cosT = np.ascontiguousarray(np.cos(emb).T).astype(BF16_NP)           # [128, T]
    sinT = np.sin(emb).T.astype(np.float32)
    sinhT = np.ascontiguousarray(
        np.concatenate([-sinT[: HD // 2], sinT[HD // 2 :]], axis=0)
    ).astype(BF16_NP)
    return cosT, sinhT


def kernel(x, Wq, Wk, Wv, Wo, **run_kwargs):
    x = np.asarray(x, dtype=np.float32)
    Wq = np.asarray(Wq, dtype=np.float32)
    Wk = np.asarray(Wk, dtype=np.float32)
    Wv = np.asarray(Wv, dtype=np.float32)
    Wo = np.asarray(Wo, dtype=np.float32)

    nc = _get_program()
    cosT, sinhT = _rope_tables()
    xT = np.ascontiguousarray(x.reshape(BT, D).T).astype(BF16_NP)  # [D, BT]
    # S^T[tk, tq] causal mask for the diagonal block: keep tq(col) >= tk(row)
    r = np.arange(128)
    triM = (r[None, :] >= r[:, None]).astype(BF16_NP)
    identM = np.eye(128, dtype=BF16_NP)

    in_maps = []
    for c in range(NCORES):
        sl = slice(c * M_PC, (c + 1) * M_PC)
        in_maps.append(
            {
                "xT": xT,
                "triT": triM,
                "identT": identM,
                "wqkT": np.ascontiguousarray(
                    np.concatenate([Wq[sl, :].T, Wk[sl, :].T], axis=1)
                ).astype(BF16_NP),
                "wvT": np.ascontiguousarray(Wv[sl, :].T).astype(BF16_NP),
                "woT": np.ascontiguousarray(Wo[:, sl].T).astype(BF16_NP),
                "cosT": cosT,
                "sinhT": sinhT,
            }
        )

    res = run_bass_kernel_spmd(nc, in_maps, list(range(NCORES)), **run_kwargs)
    acc = np.zeros((D, BT), dtype=np.float32)
    for c in range(NCORES):
        acc += res.results[c]["partialT"].astype(np.float32)
    out = np.ascontiguousarray(acc.T).reshape(B, T, D)
    if run_kwargs:
        return out, res
    return out



# revision 60
# speedup vs baseline: 1.0808x; 1.0097x over previous
"""Multi-head self-attention (B=2, T=2048, D=2048, H=16, RoPE, causal)
as a Bass/Tile kernel running SPMD on 8 trn2 NeuronCores.

Sharding: tensor-parallel over heads (2 heads per core). Each core
computes its heads' Q/K/V projections, RoPE, causal attention, and a
partial out-projection over its 256 feature columns; the host sums the
8 partial outputs (all-reduce equivalent).

Dataflow (per core, per batch):
  - x streamed per 512-wide t-block ([128, 16, 512] SBUF tiles, 4 tags);
    the first block's DMA is interleaved per-contraction-chunk with the
    weight loads so the PE starts ~2us in.
  - Q/K projections in "T-layout" (feature dim on partitions, time on
    free); RoPE rotate-half via two SBUF->SBUF partition-swap DMAs
    (sign folded into the sin table), all-bf16 combine on DVE (2x mode).
  - V projected directly in natural layout ([tk, d]): lhsT = x chunk,
    rhs = Wv slice -- no PE transposes.
  - scores computed transposed: S^T[tk, tq] per (key-chunk, q-group).
    Chunks are narrowed to the causal region (exact 136-block lower
    triangle, no fully-masked work); only the diagonal 128x128 block
    gets a mask add. The two heads' chunk streams are interleaved so
    the PE always has ~1.3us of work while exp round-trips through
    DVE/Act. Z column sums via transposed matmuls (lhsT = exp chunk,
    rhs = ones column, [128,1] output) accumulated in PSUM -- the PE
    streams 1 column per chunk instead of up-to-512.
  - normalization trails each q-group: po -> oT (unnormalized cast);
    Z columns -> DVE reciprocal -> PE transpose -> Act copy -> an
    8-descriptor DMA back into row layout; then a ones-row broadcast
    matmul (riding the score-tile PSUM ring) + in-place DVE multiply,
    paced several chunks behind so the DMA latency stays off the PE.
  - out-projection accumulates the two head-chunks in PSUM; partial
    result cast to f16 and DMA'd out; host sums partials across cores.
"""

import sys

sys.path.insert(0, "/opt/trn_rl_repo")

import ml_dtypes
import numpy as np

import concourse.bass as bass
import concourse.mybir as mybir
import concourse.tile as tile
from concourse.bass_utils import run_bass_kernel_spmd


def _legalize_waits(nc):
    """Walrus codegen rejects >2 sync waits on DMA/matmul/nop-class
    instructions, and Tile's pool-recycle waits bypass its own elision.
    Spill excess waits (>1) onto freshly inserted same-engine NoOps
    placed immediately before the offending instruction (sound w.r.t.
    per-engine program order)."""
    spill_id = [0]
    for bb in nc.m.functions[0].blocks:
        new_insts = []
        for inst in bb.instructions:
            si = getattr(inst, "sync_info", None)
            if si is None or not si.on_wait:
                new_insts.append(inst)
                continue
            eng = getattr(inst, "engine", None)
            kept = list(si.on_wait)
            if len(kept) > 1 and eng is not None:
                excess, kept = kept[:-1], kept[-1:]
                for w in excess:
                    spill_id[0] += 1
                    nop = mybir.InstNoOp(
                        name=f"I-wspill-{spill_id[0]}",
                        ins=[],
                        outs=[],
                        engine=eng,
                    )
                    nop.sync_info = mybir.SyncInfo(on_wait=[w], on_update=[])
                    new_insts.append(nop)
            if len(kept) != len(si.on_wait):
                si.on_wait[:] = kept
            new_insts.append(inst)
        if len(new_insts) != len(bb.instructions):
            bb.instructions[:] = new_insts


_PHASE_MARKS = []  # (phase_label, last_inst_index_before_phase) - profiling aid


def _mark(nc, label):
    n = -1
    for fn in nc.m.functions:
        for bb in fn.blocks:
            for ins in bb.instructions:
                if ins.name.startswith("I-"):
                    try:
                        n = max(n, int(ins.name[2:]))
                    except ValueError:
                        pass
    _PHASE_MARKS.append((label, n))


B, T, D, H, HD = 2, 2048, 2048, 16, 128
NCORES = 8
HPC = H // NCORES            # heads per core = 2
M_PC = HPC * HD              # per-core feature slice = 256
BT = B * T                   # 4096
SCALE = HD ** -0.5
ROPE_THETA = 10000.0

F32 = mybir.dt.float32
F16 = mybir.dt.float16
BF16 = mybir.dt.bfloat16
BF16_NP = ml_dtypes.bfloat16

TB = 512                     # t-block for projections / q-groups
NTB_B = T // TB              # 4 t-blocks per batch
NMC = D // 128               # 16 contraction chunks
NKC = T // 128               # 16 key chunks per batch
JPG = TB // 128              # key chunks per q-group width = 4

Copy = mybir.ActivationFunctionType.Copy
Exp = mybir.ActivationFunctionType.Exp


def build_program():
    nc = bass.Bass()

    xT_d = nc.declare_dram_parameter("xT", [D, BT], BF16, isOutput=False)
    tri_d = nc.declare_dram_parameter("triT", [128, 128], BF16, isOutput=False)
    # wq and wk concatenated so one DMA covers both (halves SP-seq time
    # on the critical startup path)
    wqk_d = nc.declare_dram_parameter(
        "wqkT", [D, 2 * M_PC], BF16, isOutput=False
    )
    wv_d = nc.declare_dram_parameter("wvT", [D, M_PC], BF16, isOutput=False)
    wo_d = nc.declare_dram_parameter("woT", [M_PC, D], BF16, isOutput=False)
    ident_d = nc.declare_dram_parameter("identT", [128, 128], BF16, isOutput=False)
    cos_d = nc.declare_dram_parameter("cosT", [HD, T], BF16, isOutput=False)
    sinh_d = nc.declare_dram_parameter("sinhT", [HD, T], BF16, isOutput=False)
    out_d = nc.declare_dram_parameter("partialT", [D, BT], F16, isOutput=True)

    xT_v = xT_d.rearrange("(c p) t -> p c t", p=128)      # [128, 16, BT]
    wqk_v = wqk_d.rearrange("(c p) n -> p c n", p=128)    # [128, 16, 512]
    wv_v = wv_d.rearrange("(c p) n -> p c n", p=128)
    wo_v = wo_d.rearrange("(c p) n -> p c n", p=128)      # [128, 2, 2048]
    out_v = out_d.rearrange("(c p) t -> p c t", p=128)    # [128, 16, BT]

    with tile.TileContext(nc) as tc:
        with (
            tc.tile_pool(name="wpool", bufs=1) as wpool,
            tc.tile_pool(name="xp", bufs=1) as xp,
            tc.tile_pool(name="big", bufs=1) as big,
            tc.tile_pool(name="rp", bufs=2) as rp,
            tc.tile_pool(name="attn_sb", bufs=8) as asb,
            tc.tile_pool(name="fs_sb", bufs=6) as fsb,
        ):
            # ---- weights + first x block, interleaved in graduated mc
            # groups (fast pipeline fill, then few big SP-cheap DMAs) ----
            wqk_sb = wpool.tile([128, NMC, 2 * M_PC], BF16, tag="wqk")
            wv_sb = wpool.tile([128, NMC, M_PC], BF16, tag="wv")
            x_tiles = {}
            xt0 = xp.tile([128, NMC, TB], BF16, tag="x0", name="x_b0_t0")
            x_tiles[(0, 0)] = xt0
            for lo, hi in ((0, 1), (1, 2), (2, 3), (3, 4), (4, 6), (6, 8),
                           (8, 10), (10, 12), (12, 14), (14, 16)):
                nc.sync.dma_start(
                    out=wqk_sb[:, lo:hi, :], in_=wqk_v[:, lo:hi, :]
                )
                # first x chunk rides the idle DVE queue, in parallel with
                # SP's weight DMA, to cut the cold-start latency
                eng = nc.scalar if lo == 0 else nc.sync
                eng.dma_start(
                    out=xt0[:, lo:hi, :], in_=xT_v[:, lo:hi, 0:TB]
                )

            cos_sb = wpool.tile([128, T], BF16, tag="cos")
            sinh_sb = wpool.tile([128, T], BF16, tag="sinh")
            nc.sync.dma_start(out=cos_sb[:, 0:TB], in_=cos_d[:, 0:TB])
            nc.sync.dma_start(out=sinh_sb[:, 0:TB], in_=sinh_d[:, 0:TB])

            def load_x(b, tb):
                t = xp.tile(
                    [128, NMC, TB], BF16, tag=f"x{tb}", name=f"x_b{b}_t{tb}"
                )
                x_tiles[(b, tb)] = t
                lo = b * T + tb * TB
                for m0 in range(0, NMC, 4):
                    nc.sync.dma_start(
                        out=t[:, m0 : m0 + 4, :],
                        in_=xT_v[:, m0 : m0 + 4, lo : lo + TB],
                    )

            # wv rides alongside tb0's V matmuls; x block 1 follows
            for m0 in range(0, NMC, 4):
                nc.sync.dma_start(
                    out=wv_sb[:, m0 : m0 + 4, :], in_=wv_v[:, m0 : m0 + 4, :]
                )
            load_x(0, 1)
            nc.sync.dma_start(out=cos_sb[:, TB:], in_=cos_d[:, TB:])
            nc.sync.dma_start(out=sinh_sb[:, TB:], in_=sinh_d[:, TB:])
            tri01 = wpool.tile([128, 128], BF16, tag="tri01")
            nc.sync.dma_start(out=tri01, in_=tri_d[:, :])
            ident = wpool.tile([128, 128], BF16, tag="ident")
            nc.sync.dma_start(out=ident, in_=ident_d[:, :])
            ones_col = wpool.tile([128, 1], BF16, tag="ones_c")
            nc.vector.memset(ones_col, 1.0)
            ones_row = wpool.tile([1, 128], BF16, tag="ones_r")
            nc.vector.memset(ones_row, 1.0)

            for tb in range(2, NTB_B):
                load_x(0, tb)

            wo_sb = wpool.tile([128, HPC, D], BF16, tag="wo")
            nc.sync.dma_start(out=wo_sb, in_=wo_v)

            for b in range(B):
                t0 = b * T  # global t offset of this batch
                _mark(nc, f"b{b}_proj")

                # persistent per-batch tensors (slots reused across b)
                qT = big.tile([128, HPC, T], BF16, tag="qT")   # [hd, h, t]
                kT = big.tile([128, HPC, T], BF16, tag="kT")
                vN = big.tile([128, NKC, M_PC], BF16, tag="vN")  # [tk, j, n]
                oT = big.tile([128, HPC, T], BF16, tag="oT")   # attn out

                # ---------------- projections + RoPE ----------------
                warm = []  # pre-emitted attention chunks
                with tc.tile_pool(name="qk_ps", bufs=1, space="PSUM") as qkp, \
                     tc.tile_pool(name="v_ps", bufs=1, space="PSUM") as vps:
                    for tb in range(NTB_B):
                        xt = x_tiles[(b, tb)]
                        ts_l = slice(tb * TB, (tb + 1) * TB)   # in-batch
                        ps = {}
                        for h in range(HPC):
                            for nm in ("q", "k"):
                                ps[nm, h] = qkp.tile(
                                    [128, TB], F32, tag=f"{nm}{h}",
                                    name=f"ps_{nm}{h}",
                                )
                        for mc in range(NMC):
                            for h in range(HPC):
                                for ni, nm in ((0, "q"), (1, "k")):
                                    hs = slice(
                                        ni * M_PC + h * HD,
                                        ni * M_PC + (h + 1) * HD,
                                    )
                                    nc.tensor.matmul(
                                        ps[nm, h],
                                        lhsT=wqk_sb[:, mc, hs],
                                        rhs=xt[:, mc, :],
                                        start=(mc == 0),
                                        stop=(mc == NMC - 1),
                                    )
                        # raw bf16 casts early on Act (deps ready now)
                        raws = {}
                        for nm in ("q", "k"):
                            for h in range(HPC):
                                raw = rp.tile(
                                    [128, TB], BF16, tag=f"raw{nm}{h}"
                                )
                                nc.scalar.activation(raw, ps[nm, h], Copy)
                                raws[nm, h] = raw
                        # RoPE before V: the qk/sw PSUM banks drain while the
                        # PE streams V matmuls, so the next phase's bank-WAR
                        # waits resolve before the PE gets there
                        for nm, dest in (("q", qT), ("k", kT)):
                            for h in range(HPC):
                                raw = raws[nm, h]
                                swb = rp.tile([128, TB], BF16, tag="swb")
                                nc.sync.dma_start(
                                    out=swb[0:64, :], in_=raw[64:128, :]
                                )
                                nc.sync.dma_start(
                                    out=swb[64:128, :], in_=raw[0:64, :]
                                )
                                t2 = rp.tile([128, TB], BF16, tag="t2")
                                nc.vector.tensor_mul(t2, swb, sinh_sb[:, ts_l])
                                t1 = rp.tile([128, TB], BF16, tag="t1")
                                nc.vector.tensor_mul(t1, raw, cos_sb[:, ts_l])
                                nc.vector.tensor_add(dest[:, h, ts_l], t1, t2)
                        # V directly in natural layout: per 128-wide tk chunk
                        def v_chunk(s):
                            j = tb * JPG + s
                            pv = vps.tile(
                                [128, M_PC], F32, tag=f"v{s % 2}",
                                name=f"ps_v{s % 2}",
                            )
                            for mc in range(NMC):
                                nc.tensor.matmul(
                                    pv,
                                    lhsT=xt[:, mc, s * 128 : (s + 1) * 128],
                                    rhs=wv_sb[:, mc, :],
                                    start=(mc == 0),
                                    stop=(mc == NMC - 1),
                                )
                            nc.scalar.activation(vN[:, j, :], pv, Copy)

                        for s in range(JPG):
                            # tb1's tail / tb2 / tb3 V chunks are deferred
                            # into the attention chunks (the window is
                            # Act-bound there; these give the PE ~1.7us
                            # each and move their Act/DVE load out of the
                            # backlog)
                            if tb == 0 or (tb == 1 and s < 2):
                                v_chunk(s)
                            # between tb3's V chunks, warm up attention: the
                            # (qg0, j0) score pair + exp run in drained qk
                            # PSUM slots, their latency hidden by V matmuls,
                            # so the attention loop starts with a full
                            # pipeline instead of a fill bubble
                            if tb == NTB_B - 1 and s in (1, 2):
                                # chunk (0,0) after V s1 in the q-tag slots,
                                # chunk (0,1) after V s2 in the k-tag slots:
                                # the attention loop starts two chunks deep,
                                # matching the Z trail exactly
                                wj = 0 if s == 1 else 1
                                woff = 0 if wj == 0 else 128
                                wtag = "q" if s == 1 else "k"
                                wpt2 = asb.tile(
                                    [128, HPC, TB], BF16, tag="pt2",
                                    name="warm_pt",
                                )
                                for h in range(HPC):
                                    wst = qkp.tile(
                                        [128, TB], F32,
                                        tag=f"{wtag}{h}", name="warm_st",
                                    )
                                    nc.tensor.matmul(
                                        wst[:, woff:],
                                        lhsT=kT[:, h, wj * 128 : (wj + 1) * 128],
                                        rhs=qT[:, h, woff:TB],
                                        start=True,
                                        stop=True,
                                    )
                                    nc.scalar.activation(
                                        wpt2[:, h, woff:], wst[:, woff:],
                                        Exp, scale=SCALE,
                                    )
                                    nc.vector.tensor_mul(
                                        wpt2[:, h, woff : woff + 128],
                                        wpt2[:, h, woff : woff + 128],
                                        tri01,
                                    )
                                warm.append([0, wj, (wpt2, woff), False])



                _mark(nc, f"b{b}_attn")
                # ---------------- attention + out-projection ----------------
                # Both heads' score tiles share one 2-bank PSUM tile so a
                # single exp call covers them (Act is the scarce engine in
                # this window). Z/PV trail the score/exp stream by 2/3
                # chunks, software-pipelined ACROSS q-groups. Z is
                # accumulated as [128,1] columns via transposed matmuls
                # (exp chunk stationary, ones moving) -- ~1 PE row per chunk
                # instead of up-to-512 -- and routed back to row layout off
                # the PE (recip -> PE transpose -> Act copy -> 8-descriptor
                # DMA). The out-projection interleaves with the chunk
                # stream, one 128-column block per chunk once its q-group is
                # normalized, keeping the PE fed while Act drains exps; the
                # leftovers plus the last group's norms form the batch tail
                # on the score-tile ring.
                with tc.tile_pool(name="st_ps", bufs=2, space="PSUM") as stp, \
                     tc.tile_pool(name="pv_ps", bufs=1, space="PSUM") as pvp, \
                     tc.tile_pool(name="z_ps", bufs=1, space="PSUM") as zpp, \
                     tc.tile_pool(name="fo_ps", bufs=1, space="PSUM") as fop:
                    norm_q = []  # pending [qg, h, ready_at_pv_call]
                    zr_rows = {}  # qg -> [1, HPC*JPG, 128] recip-Z row tile
                    zr_pend = []  # [qg, zrcol, z_call_at_emit]
                    ob_q = []  # (tb, nb) out-proj blocks ready to emit
                    fs_state = {}  # tb -> current fs store-group tile
                    n_z = [0]
                    n_pv = [0]

                    def norm_step(pool, tag):
                        nqg, h, _ = norm_q.pop(0)
                        qs0 = nqg * TB
                        zbp = pool.tile([128, TB], F32, tag=tag, name="zbp")
                        nc.tensor.matmul(
                            zbp,
                            lhsT=ones_row,
                            rhs=zr_rows[nqg][0:1, h * JPG : (h + 1) * JPG, :],
                            start=True,
                            stop=True,
                        )
                        nc.vector.tensor_mul(
                            oT[:, h, qs0 : qs0 + TB],
                            oT[:, h, qs0 : qs0 + TB],
                            zbp,
                        )
                        if h == HPC - 1:  # group fully normalized
                            ob_q.extend(
                                (nqg, nb) for nb in range(D // 128)
                            )

                    def emit_block(btb, nb, pool, tag):
                        """One 128-column out-projection block: 2 matmuls
                        into a borrowed PSUM slot, cast into the current
                        4-block store group, DMA when the group completes."""
                        tbs = slice(btb * TB, (btb + 1) * TB)
                        nbs = slice(nb * 128, (nb + 1) * 128)
                        fo = pool.tile([128, TB], F32, tag=tag, name="fo")
                        for m in range(HPC):
                            nc.tensor.matmul(
                                fo,
                                lhsT=wo_sb[:, m, nbs],
                                rhs=oT[:, m, tbs],
                                start=(m == 0),
                                stop=(m == HPC - 1),
                            )
                        if b == B - 1 and btb == NTB_B - 1 and nb >= 12:
                            # shrink the final stores: the last DMA's
                            # latency is the kernel's tail
                            grp = 2 if nb < 14 else 1
                        else:
                            grp = 4
                        if nb % grp == 0:
                            fs_state[btb] = fsb.tile(
                                [128, 4, TB], F16, tag="fs", name="fs"
                            )
                        fs = fs_state[btb]
                        # 9/16 of casts on Act (DVE also carries the norm
                        # muls and zr copies); the kernel's final two casts
                        # go to Act, whose queue is empty at the end
                        on_act = nb % 2 == 0 or nb % 16 == 7 or grp == 1
                        if on_act:
                            nc.scalar.activation(fs[:, nb % grp, :], fo, Copy)
                        else:
                            nc.vector.tensor_copy(fs[:, nb % grp, :], fo)
                        if nb % grp == grp - 1:
                            # stores alternate between the Pool and SP
                            # queues (either alone serializes ~1.6us per
                            # store, which would gate the batch tail); the
                            # kernel's very last store goes to the Act
                            # queue, which is empty at the end
                            last = (
                                b == B - 1
                                and btb == NTB_B - 1
                                and nb == D // 128 - 1
                            )
                            if last:
                                deng = nc.scalar
                            elif (btb * 16 + nb) % 8 < 4:
                                deng = nc.gpsimd
                            else:
                                deng = nc.sync
                            deng.dma_start(
                                out=out_v[
                                    :,
                                    nb - grp + 1 : nb + 1,
                                    t0 + btb * TB : t0 + (btb + 1) * TB,
                                ],
                                in_=fs[:, 0:grp, :],
                            )

                    def process_zr():
                        """Late half of the Z path: PE-transpose the recip-Z
                        columns to [8,128] (riding the out-proj PSUM slot),
                        copy PSUM->SBUF on Act, and DMA the 8 rows into a
                        single [1, 8*128] row for the broadcast matmul."""
                        zqg, zrcol, _ = zr_pend.pop(0)
                        zrT = fop.tile(
                            [HPC * JPG, 128], BF16, tag="foA", name="zrT"
                        )
                        nc.tensor.transpose(zrT, zrcol, ident)
                        zrS = rp.tile([HPC * JPG, 128], BF16, tag="zs8")
                        # DVE, not Act: at the batch tail Act still has the
                        # trailing exps queued, which would delay the DMA
                        nc.vector.tensor_copy(zrS, zrT)
                        zrow_t = rp.tile(
                            [1, HPC * JPG, 128], BF16, tag="zrow"
                        )
                        # idle Pool queue: the SP queue carries x loads and
                        # output stores, which would delay this tiny DMA
                        nc.gpsimd.dma_start(out=zrow_t, in_=zrS)
                        zr_rows[zqg] = zrow_t
                        for h in range(HPC):
                            norm_q.append([zqg, h, n_pv[0] + 5 + 2 * h])

                    # (qg, j) chunk schedule, flattened; (0,0) was
                    # pre-warmed inside the projection scope
                    sched = [
                        (qg, j)
                        for qg in range(NTB_B)
                        for j in range(JPG * (qg + 1))
                    ][2:]
                    po = {}
                    zcol = {}
                    pend = warm  # [qg, j, (pt2, off), z_done]

                    def z_step(pqg, pj, pts):
                        """Z partial sums (trail 2) as transposed matmuls:
                        one [128,1] PSUM column per (head, 128-wide tq sub),
                        accumulated over key chunks. Each column's stop fires
                        on its last causal chunk; the recip covers all 8
                        columns in one DVE op on the group's last chunk."""
                        n_z[0] += 1
                        if zr_pend and n_z[0] - zr_pend[0][2] >= 1:
                            process_zr()
                        base = JPG * pqg
                        if pqg not in zcol:
                            zcol[pqg] = zpp.tile(
                                [128, HPC * JPG], F32, tag="zc", name="zc"
                            )
                        zc = zcol[pqg]
                        pt2, offp = pts
                        sub0 = max(0, pj - base)
                        # all 8 columns share ONE accumulation group (PSUM
                        # groups are tracked per 2KB zero region = the whole
                        # bank): start only on the group's first matmul --
                        # its start bit marks the bank pending-zero, so the
                        # other columns' first writes see zeros -- and stop
                        # only on its very last
                        for h in range(HPC):
                            for s in range(sub0, JPG):
                                nc.tensor.matmul(
                                    zc[:, h * JPG + s : h * JPG + s + 1],
                                    lhsT=pt2[:, h, s * 128 : (s + 1) * 128],
                                    rhs=ones_col,
                                    start=(pj == 0 and h == 0 and s == 0),
                                    stop=(
                                        pj == JPG * (pqg + 1) - 1
                                        and h == HPC - 1
                                        and s == JPG - 1
                                    ),
                                )
                        if pj == JPG * (pqg + 1) - 1:
                            zrcol = rp.tile(
                                [128, HPC * JPG], BF16, tag="zrc"
                            )
                            with nc.allow_low_precision(
                                reason="bf16 1/Z: 0.4% rel, in tolerance"
                            ):
                                nc.vector.reciprocal(zrcol, zc)
                            zr_pend.append([pqg, zrcol, n_z[0]])

                    def pv_step(pqg, pj, pts):
                        """PV matmuls (trail 3); one paced 1/Z broadcast may
                        ride along per call once its DMA has had >= 5 chunks
                        to land. Returns True if a broadcast was emitted."""
                        n_pv[0] += 1
                        popped = False
                        if norm_q and norm_q[0][2] <= n_pv[0]:
                            norm_step(fop, "foA")
                            popped = True
                        pjmax = JPG * (pqg + 1)
                        if (pqg, 0) not in po:
                            for h in range(HPC):
                                po[pqg, h] = pvp.tile(
                                    [128, TB], F32, tag=f"po{h}", name=f"po{h}"
                                )
                        pt2, offp = pts
                        for h in range(HPC):
                            nc.tensor.matmul(
                                po[pqg, h][:, offp:],
                                lhsT=vN[:, pj, h * HD : (h + 1) * HD],
                                rhs=pt2[:, h, offp:],
                                start=(pj == 0),
                                stop=(pj == pjmax - 1),
                            )
                        if pj == pjmax - 1:
                            qs0 = pqg * TB
                            for h in range(HPC):
                                # split across Act/DVE: the next group's po
                                # bank reuse waits on these
                                if h == 0:
                                    nc.scalar.activation(
                                        oT[:, h, qs0 : qs0 + TB],
                                        po[pqg, h],
                                        Copy,
                                    )
                                else:
                                    nc.vector.tensor_copy(
                                        oT[:, h, qs0 : qs0 + TB], po[pqg, h]
                                    )
                        return popped

                    # deferred V chunks: tb2's early (its vN rows are needed
                    # by group 2's PV), tb3's late; each takes the first
                    # conflict-free foA slot at/after its target chunk
                    defer_v = [(NTB_B - 2, s, (0, 1, 7, 11)[s]) for s in range(JPG)]
                    defer_v += [(1, 2, 4), (1, 3, 6)]
                    defer_v += [(NTB_B - 1, s, (15, 19, 23, 27)[s]) for s in range(JPG)]
                    defer_v.sort(key=lambda e: e[2])

                    def emit_v(vtb, s):
                        jv = vtb * JPG + s
                        xt_v_src = x_tiles[(b, vtb)]
                        pv = fop.tile(
                            [128, M_PC], F32, tag="foA", name="ps_vd"
                        )
                        for mc in range(NMC):
                            nc.tensor.matmul(
                                pv,
                                lhsT=xt_v_src[:, mc, s * 128 : (s + 1) * 128],
                                rhs=wv_sb[:, mc, :],
                                start=(mc == 0),
                                stop=(mc == NMC - 1),
                            )
                        nc.vector.tensor_copy(vN[:, jv, :], pv)

                    for n_ch, (qg, j) in enumerate(sched):
                        qs0 = qg * TB
                        off = max(0, (j - JPG * qg) * 128)
                        ks = slice(j * 128, (j + 1) * 128)
                        st2 = stp.tile([128, HPC, TB], F32, tag="st2")
                        for h in range(HPC):
                            nc.tensor.matmul(
                                st2[:, h, off:],
                                lhsT=kT[:, h, ks],
                                rhs=qT[:, h, qs0 + off : qs0 + TB],
                                start=True,
                                stop=True,
                            )
                        # exp first (it's on the PE's st2-ring critical
                        # path), then zero the diagonal block's masked wedge
                        # on the SBUF side where the 2-chunk Z/PV trail
                        # hides the DVE latency
                        pt2 = asb.tile([128, HPC, TB], BF16, tag="pt2")
                        nc.scalar.activation(
                            pt2[:, :, off:], st2[:, :, off:], Exp, scale=SCALE
                        )
                        pend.append([qg, j, (pt2, off), False])
                        popped = False
                        if len(pend) >= 4:
                            it = pend.pop(0)
                            popped = pv_step(it[0], it[1], it[2])
                        # masked-wedge zeroing emitted after pv_step: the
                        # group-end oT copies get DVE queue priority (the
                        # wedge isn't read until the Z step 2 chunks out)
                        if j >= JPG * qg:
                            for h in range(HPC):
                                nc.vector.tensor_mul(
                                    pt2[:, h, off : off + 128],
                                    pt2[:, h, off : off + 128],
                                    tri01,
                                )
                        if (
                            defer_v
                            and n_ch >= defer_v[0][2]
                            and not popped
                            and not zr_pend
                        ):
                            vtb, vs, _ = defer_v.pop(0)
                            emit_v(vtb, vs)
                        if b == 0 and n_ch == 10:
                            # b1's x loads: emitted once b0's deferred V
                            # matmuls (the last xt readers) are in the stream
                            for ltb in range(NTB_B):
                                load_x(1, ltb)
                        if len(pend) >= 3:
                            it = pend[-3]
                            z_step(it[0], it[1], it[2])
                            it[3] = True
                    for it in pend:
                        if not it[3]:
                            z_step(it[0], it[1], it[2])
                    while pend:
                        it = pend.pop(0)
                        pv_step(it[0], it[1], it[2])
                    # batch tail: the last group's Z path (transpose after
                    # the PV flush so the recip has landed), then leftover
                    # out-proj blocks interleaved with the remaining 1/Z
                    # broadcasts on the 2-bank score ring -- blocks ahead of
                    # each norm cover the zr DMA latency.
                    while zr_pend:
                        process_zr()
                    _mark(nc, f"b{b}_outproj")
                    # tail: all remaining out-proj blocks, rotating over
                    # three idle PSUM slots (half an st2 chunk slot + the
                    # two drained po banks) so each block's WAR lands on a
                    # cast three blocks back; the last group's norms pop
                    # between batches of four, after enough blocks have
                    # covered the zr DMA latency.
                    rot = [
                        (stp, "st2"),
                        (pvp, "po0"),
                        (pvp, "po1"),
                        (fop, "foA"),
                        (zpp, "zc"),
                    ]
                    r = [0]

                    def tail_slot():
                        p, tg = rot[r[0] % len(rot)]
                        r[0] += 1
                        return p, tg

                    first = True
                    while ob_q or norm_q:
                        k = 0
                        while ob_q and k < (8 if first else 4):
                            emit_block(*ob_q.pop(0), *tail_slot())
                            k += 1
                        first = False
                        if norm_q:
                            norm_step(*tail_slot())
    _legalize_waits(nc)
    return nc


_NC_CACHE = None


def _get_program():
    global _NC_CACHE
    if _NC_CACHE is None:
        _NC_CACHE = build_program()
    return _NC_CACHE


def _rope_tables():
    inv_freq = 1.0 / (ROPE_THETA ** (np.arange(0, HD, 2, dtype=np.float32) / HD))
    freqs = np.arange(T, dtype=np.float32)[:, None] * inv_freq[None, :]  # (T, 64)
    emb = np.concatenate([freqs, freqs], axis=-1)                        # (T, 128)
    cosT = np.ascontiguousarray(np.cos(emb).T).astype(BF16_NP)           # [128, T]
    sinT = np.sin(emb).T.astype(np.float32)
    sinhT = np.ascontiguousarray(
        np.concatenate([-sinT[: HD // 2], sinT[HD // 2 :]], axis=0)
    ).astype(BF16_NP)
    return cosT, sinhT


def kernel(x, Wq, Wk, Wv, Wo, **run_kwargs):
    x = np.asarray(x, dtype=np.float32)
    Wq = np.asarray(Wq, dtype=np.float32)
    Wk = np.asarray(Wk, dtype=np.float32)
    Wv = np.asarray(Wv, dtype=np.float32)
    Wo = np.asarray(Wo, dtype=np.float32)

    nc = _get_program()
    cosT, sinhT = _rope_tables()
    xT = np.ascontiguousarray(x.reshape(BT, D).T).astype(BF16_NP)  # [D, BT]
    # S^T[tk, tq] causal mask for the diagonal block: keep tq(col) >= tk(row)
    r = np.arange(128)
    triM = (r[None, :] >= r[:, None]).astype(BF16_NP)
    identM = np.eye(128, dtype=BF16_NP)

    in_maps = []
    for c in range(NCORES):
        sl = slice(c * M_PC, (c + 1) * M_PC)
        in_maps.append(
            {
                "xT": xT,
                "triT": triM,
                "identT": identM,
                "wqkT": np.ascontiguousarray(
                    np.concatenate([Wq[sl, :].T, Wk[sl, :].T], axis=1)
                ).astype(BF16_NP),
                "wvT": np.ascontiguousarray(Wv[sl, :].T).astype(BF16_NP),
                "woT": np.ascontiguousarray(Wo[:, sl].T).astype(BF16_NP),
                "cosT": cosT,
                "sinhT": sinhT,
            }
        )

    res = run_bass_kernel_spmd(nc, in_maps, list(range(NCORES)), **run_kwargs)
    acc = np.zeros((D, BT), dtype=np.float32)
    for c in range(NCORES):
        acc += res.results[c]["partialT"].astype(np.float32)
    out = np.ascontiguousarray(acc.T).reshape(B, T, D)
    if run_kwargs:
        return out, res
    return out



# revision 66
# speedup vs baseline: 1.0832x; 1.0022x over previous
"""Multi-head self-attention (B=2, T=2048, D=2048, H=16, RoPE, causal)
as a Bass/Tile kernel running SPMD on 8 trn2 NeuronCores.

Sharding: tensor-parallel over heads (2 heads per core). Each core
computes its heads' Q/K/V projections, RoPE, causal attention, and a
partial out-projection over its 256 feature columns; the host sums the
8 partial outputs (all-reduce equivalent).

Dataflow (per core, per batch):
  - x streamed per 512-wide t-block ([128, 16, 512] SBUF tiles, 4 tags);
    the first block's DMA is interleaved per-contraction-chunk with the
    weight loads so the PE starts ~2us in.
  - Q/K projections in "T-layout" (feature dim on partitions, time on
    free); RoPE rotate-half via two SBUF->SBUF partition-swap DMAs
    (sign folded into the sin table), all-bf16 combine on DVE (2x mode).
  - V projected directly in natural layout ([tk, d]): lhsT = x chunk,
    rhs = Wv slice -- no PE transposes.
  - scores computed transposed: S^T[tk, tq] per (key-chunk, q-group).
    Chunks are narrowed to the causal region (exact 136-block lower
    triangle, no fully-masked work); only the diagonal 128x128 block
    gets a mask add. The two heads' chunk streams are interleaved so
    the PE always has ~1.3us of work while exp round-trips through
    DVE/Act. Z column sums via transposed matmuls (lhsT = exp chunk,
    rhs = ones column, [128,1] output) accumulated in PSUM -- the PE
    streams 1 column per chunk instead of up-to-512.
  - normalization trails each q-group: po -> oT (unnormalized cast);
    Z columns -> DVE reciprocal -> PE transpose -> Act copy -> an
    8-descriptor DMA back into row layout; then a ones-row broadcast
    matmul (riding the score-tile PSUM ring) + in-place DVE multiply,
    paced several chunks behind so the DMA latency stays off the PE.
  - out-projection accumulates the two head-chunks in PSUM; partial
    result cast to f16 and DMA'd out; host sums partials across cores.
"""

import sys

sys.path.insert(0, "/opt/trn_rl_repo")

import ml_dtypes
import numpy as np

import concourse.bass as bass
import concourse.mybir as mybir
import concourse.tile as tile
from concourse.bass_utils import run_bass_kernel_spmd


def _legalize_waits(nc):
    """Walrus codegen rejects >2 sync waits on DMA/matmul/nop-class
    instructions, and Tile's pool-recycle waits bypass its own elision.
    Spill excess waits (>1) onto freshly inserted same-engine NoOps
    placed immediately before the offending instruction (sound w.r.t.
    per-engine program order)."""
    spill_id = [0]
    for bb in nc.m.functions[0].blocks:
        new_insts = []
        for inst in bb.instructions:
            si = getattr(inst, "sync_info", None)
            if si is None or not si.on_wait:
                new_insts.append(inst)
                continue
            eng = getattr(inst, "engine", None)
            kept = list(si.on_wait)
            if len(kept) > 1 and eng is not None:
                excess, kept = kept[:-1], kept[-1:]
                for w in excess:
                    spill_id[0] += 1
                    nop = mybir.InstNoOp(
                        name=f"I-wspill-{spill_id[0]}",
                        ins=[],
                        outs=[],
                        engine=eng,
                    )
                    nop.sync_info = mybir.SyncInfo(on_wait=[w], on_update=[])
                    new_insts.append(nop)
            if len(kept) != len(si.on_wait):
                si.on_wait[:] = kept
            new_insts.append(inst)
        if len(new_insts) != len(bb.instructions):
            bb.instructions[:] = new_insts


_PHASE_MARKS = []  # (phase_label, last_inst_index_before_phase) - profiling aid


def _mark(nc, label):
    n = -1
    for fn in nc.m.functions:
        for bb in fn.blocks:
            for ins in bb.instructions:
                if ins.name.startswith("I-"):
                    try:
                        n = max(n, int(ins.name[2:]))
                    except ValueError:
                        pass
    _PHASE_MARKS.append((label, n))


B, T, D, H, HD = 2, 2048, 2048, 16, 128
NCORES = 8
HPC = H // NCORES            # heads per core = 2
M_PC = HPC * HD              # per-core feature slice = 256
BT = B * T                   # 4096
SCALE = HD ** -0.5
ROPE_THETA = 10000.0

F32 = mybir.dt.float32
F16 = mybir.dt.float16
BF16 = mybir.dt.bfloat16
BF16_NP = ml_dtypes.bfloat16

TB = 512                     # t-block for projections / q-groups
NTB_B = T // TB              # 4 t-blocks per batch
NMC = D // 128               # 16 contraction chunks
NKC = T // 128               # 16 key chunks per batch
JPG = TB // 128              # key chunks per q-group width = 4

Copy = mybir.ActivationFunctionType.Copy
Exp = mybir.ActivationFunctionType.Exp


def build_program():
    nc = bass.Bass()

    xT_d = nc.declare_dram_parameter("xT", [D, BT], BF16, isOutput=False)
    tri_d = nc.declare_dram_parameter("triT", [128, 128], BF16, isOutput=False)
    # wq and wk concatenated so one DMA covers both (halves SP-seq time
    # on the critical startup path)
    wqk_d = nc.declare_dram_parameter(
        "wqkT", [D, 2 * M_PC], BF16, isOutput=False
    )
    wv_d = nc.declare_dram_parameter("wvT", [D, M_PC], BF16, isOutput=False)
    wo_d = nc.declare_dram_parameter("woT", [M_PC, D], BF16, isOutput=False)
    ident_d = nc.declare_dram_parameter("identT", [128, 128], BF16, isOutput=False)
    cos_d = nc.declare_dram_parameter("cosT", [HD, T], BF16, isOutput=False)
    sinh_d = nc.declare_dram_parameter("sinhT", [HD, T], BF16, isOutput=False)
    out_d = nc.declare_dram_parameter("partialT", [D, BT], F16, isOutput=True)

    xT_v = xT_d.rearrange("(c p) t -> p c t", p=128)      # [128, 16, BT]
    wqk_v = wqk_d.rearrange("(c p) n -> p c n", p=128)    # [128, 16, 512]
    wv_v = wv_d.rearrange("(c p) n -> p c n", p=128)
    wo_v = wo_d.rearrange("(c p) n -> p c n", p=128)      # [128, 2, 2048]
    out_v = out_d.rearrange("(c p) t -> p c t", p=128)    # [128, 16, BT]

    with tile.TileContext(nc) as tc:
        with (
            tc.tile_pool(name="wpool", bufs=1) as wpool,
            tc.tile_pool(name="xp", bufs=1) as xp,
            tc.tile_pool(name="big", bufs=1) as big,
            tc.tile_pool(name="rp", bufs=2) as rp,
            tc.tile_pool(name="attn_sb", bufs=8) as asb,
            tc.tile_pool(name="fs_sb", bufs=6) as fsb,
        ):
            # ---- weights + first x block, interleaved in graduated mc
            # groups (fast pipeline fill, then few big SP-cheap DMAs) ----
            wqk_sb = wpool.tile([128, NMC, 2 * M_PC], BF16, tag="wqk")
            wv_sb = wpool.tile([128, NMC, M_PC], BF16, tag="wv")
            x_tiles = {}
            xt0 = xp.tile([128, NMC, TB], BF16, tag="x0", name="x_b0_t0")
            x_tiles[(0, 0)] = xt0
            for lo, hi in ((0, 1), (1, 2), (2, 3), (3, 4), (4, 6), (6, 8),
                           (8, 10), (10, 12), (12, 14), (14, 16)):
                nc.sync.dma_start(
                    out=wqk_sb[:, lo:hi, :], in_=wqk_v[:, lo:hi, :]
                )
                # first x chunk rides the idle DVE queue, in parallel with
                # SP's weight DMA, to cut the cold-start latency
                eng = nc.scalar if lo == 0 else nc.sync
                eng.dma_start(
                    out=xt0[:, lo:hi, :], in_=xT_v[:, lo:hi, 0:TB]
                )

            cos_sb = wpool.tile([128, T], BF16, tag="cos")
            sinh_sb = wpool.tile([128, T], BF16, tag="sinh")
            nc.sync.dma_start(out=cos_sb[:, 0:TB], in_=cos_d[:, 0:TB])
            nc.sync.dma_start(out=sinh_sb[:, 0:TB], in_=sinh_d[:, 0:TB])

            def load_x(b, tb):
                t = xp.tile(
                    [128, NMC, TB], BF16, tag=f"x{tb}", name=f"x_b{b}_t{tb}"
                )
                x_tiles[(b, tb)] = t
                lo = b * T + tb * TB
                for m0 in range(0, NMC, 4):
                    nc.sync.dma_start(
                        out=t[:, m0 : m0 + 4, :],
                        in_=xT_v[:, m0 : m0 + 4, lo : lo + TB],
                    )

            # wv rides alongside tb0's V matmuls; x block 1 follows
            for m0 in range(0, NMC, 4):
                nc.sync.dma_start(
                    out=wv_sb[:, m0 : m0 + 4, :], in_=wv_v[:, m0 : m0 + 4, :]
                )
            load_x(0, 1)
            nc.sync.dma_start(out=cos_sb[:, TB:], in_=cos_d[:, TB:])
            nc.sync.dma_start(out=sinh_sb[:, TB:], in_=sinh_d[:, TB:])
            tri01 = wpool.tile([128, 128], BF16, tag="tri01")
            nc.sync.dma_start(out=tri01, in_=tri_d[:, :])
            ident = wpool.tile([128, 128], BF16, tag="ident")
            nc.sync.dma_start(out=ident, in_=ident_d[:, :])
            ones_col = wpool.tile([128, 1], BF16, tag="ones_c")
            nc.vector.memset(ones_col, 1.0)
            ones_row = wpool.tile([1, 128], BF16, tag="ones_r")
            nc.vector.memset(ones_row, 1.0)

            for tb in range(2, NTB_B):
                load_x(0, tb)

            wo_sb = wpool.tile([128, HPC, D], BF16, tag="wo")
            nc.sync.dma_start(out=wo_sb, in_=wo_v)

            for b in range(B):
                t0 = b * T  # global t offset of this batch
                _mark(nc, f"b{b}_proj")

                # persistent per-batch tensors (slots reused across b)
                qT = big.tile([128, HPC, T], BF16, tag="qT")   # [hd, h, t]
                kT = big.tile([128, HPC, T], BF16, tag="kT")
                vN = big.tile([128, NKC, M_PC], BF16, tag="vN")  # [tk, j, n]
                oT = big.tile([128, HPC, T], BF16, tag="oT")   # attn out

                # ---------------- projections + RoPE ----------------
                warm = []  # pre-emitted attention chunks
                with tc.tile_pool(name="qk_ps", bufs=1, space="PSUM") as qkp, \
                     tc.tile_pool(name="v_ps", bufs=1, space="PSUM") as vps:
                    for tb in range(NTB_B):
                        xt = x_tiles[(b, tb)]
                        ts_l = slice(tb * TB, (tb + 1) * TB)   # in-batch
                        ps = {}
                        for h in range(HPC):
                            for nm in ("q", "k"):
                                ps[nm, h] = qkp.tile(
                                    [128, TB], F32, tag=f"{nm}{h}",
                                    name=f"ps_{nm}{h}",
                                )
                        for mc in range(NMC):
                            for h in range(HPC):
                                for ni, nm in ((0, "q"), (1, "k")):
                                    hs = slice(
                                        ni * M_PC + h * HD,
                                        ni * M_PC + (h + 1) * HD,
                                    )
                                    nc.tensor.matmul(
                                        ps[nm, h],
                                        lhsT=wqk_sb[:, mc, hs],
                                        rhs=xt[:, mc, :],
                                        start=(mc == 0),
                                        stop=(mc == NMC - 1),
                                    )
                        # raw bf16 casts early on Act (deps ready now)
                        raws = {}
                        for nm in ("q", "k"):
                            for h in range(HPC):
                                raw = rp.tile(
                                    [128, TB], BF16, tag=f"raw{nm}{h}"
                                )
                                nc.scalar.activation(raw, ps[nm, h], Copy)
                                raws[nm, h] = raw
                        # RoPE before V: the qk/sw PSUM banks drain while the
                        # PE streams V matmuls, so the next phase's bank-WAR
                        # waits resolve before the PE gets there
                        for nm, dest in (("q", qT), ("k", kT)):
                            for h in range(HPC):
                                raw = raws[nm, h]
                                swb = rp.tile([128, TB], BF16, tag="swb")
                                nc.sync.dma_start(
                                    out=swb[0:64, :], in_=raw[64:128, :]
                                )
                                nc.sync.dma_start(
                                    out=swb[64:128, :], in_=raw[0:64, :]
                                )
                                t2 = rp.tile([128, TB], BF16, tag="t2")
                                nc.vector.tensor_mul(t2, swb, sinh_sb[:, ts_l])
                                t1 = rp.tile([128, TB], BF16, tag="t1")
                                nc.vector.tensor_mul(t1, raw, cos_sb[:, ts_l])
                                nc.vector.tensor_add(dest[:, h, ts_l], t1, t2)
                        # V directly in natural layout: per 128-wide tk chunk
                        def v_chunk(s):
                            j = tb * JPG + s
                            pv = vps.tile(
                                [128, M_PC], F32, tag=f"v{s % 2}",
                                name=f"ps_v{s % 2}",
                            )
                            for mc in range(NMC):
                                nc.tensor.matmul(
                                    pv,
                                    lhsT=xt[:, mc, s * 128 : (s + 1) * 128],
                                    rhs=wv_sb[:, mc, :],
                                    start=(mc == 0),
                                    stop=(mc == NMC - 1),
                                )
                            nc.scalar.activation(vN[:, j, :], pv, Copy)

                        for s in range(JPG):
                            # tb1's tail / tb2 / tb3 V chunks are deferred
                            # into the attention chunks (the window is
                            # Act-bound there; these give the PE ~1.7us
                            # each and move their Act/DVE load out of the
                            # backlog)
                            if tb == 0 or (tb == 1 and s < 2):
                                v_chunk(s)
                            # between tb3's V chunks, warm up attention: the
                            # (qg0, j0) score pair + exp run in drained qk
                            # PSUM slots, their latency hidden by V matmuls,
                            # so the attention loop starts with a full
                            # pipeline instead of a fill bubble
                            if tb == NTB_B - 1 and s in (1, 2):
                                # chunk (0,0) after V s1 in the q-tag slots,
                                # chunk (0,1) after V s2 in the k-tag slots:
                                # the attention loop starts two chunks deep,
                                # matching the Z trail exactly
                                wj = 0 if s == 1 else 1
                                woff = 0 if wj == 0 else 128
                                wtag = "q" if s == 1 else "k"
                                wpt2 = asb.tile(
                                    [128, HPC, TB], BF16, tag="pt2",
                                    name="warm_pt",
                                )
                                for h in range(HPC):
                                    wst = qkp.tile(
                                        [128, TB], F32,
                                        tag=f"{wtag}{h}", name="warm_st",
                                    )
                                    nc.tensor.matmul(
                                        wst[:, woff:],
                                        lhsT=kT[:, h, wj * 128 : (wj + 1) * 128],
                                        rhs=qT[:, h, woff:TB],
                                        start=True,
                                        stop=True,
                                    )
                                    nc.scalar.activation(
                                        wpt2[:, h, woff:], wst[:, woff:],
                                        Exp, scale=SCALE,
                                    )
                                    nc.vector.tensor_mul(
                                        wpt2[:, h, woff : woff + 128],
                                        wpt2[:, h, woff : woff + 128],
                                        tri01,
                                    )
                                warm.append([0, wj, (wpt2, woff), False])



                _mark(nc, f"b{b}_attn")
                # ---------------- attention + out-projection ----------------
                # Both heads' score tiles share one 2-bank PSUM tile so a
                # single exp call covers them (Act is the scarce engine in
                # this window). Z/PV trail the score/exp stream by 2/3
                # chunks, software-pipelined ACROSS q-groups. Z is
                # accumulated as [128,1] columns via transposed matmuls
                # (exp chunk stationary, ones moving) -- ~1 PE row per chunk
                # instead of up-to-512 -- and routed back to row layout off
                # the PE (recip -> PE transpose -> Act copy -> 8-descriptor
                # DMA). The out-projection interleaves with the chunk
                # stream, one 128-column block per chunk once its q-group is
                # normalized, keeping the PE fed while Act drains exps; the
                # leftovers plus the last group's norms form the batch tail
                # on the score-tile ring.
                with tc.tile_pool(name="st_ps", bufs=2, space="PSUM") as stp, \
                     tc.tile_pool(name="pv_ps", bufs=1, space="PSUM") as pvp, \
                     tc.tile_pool(name="z_ps", bufs=1, space="PSUM") as zpp, \
                     tc.tile_pool(name="fo_ps", bufs=1, space="PSUM") as fop:
                    norm_q = []  # pending [qg, h, ready_at_pv_call]
                    zr_rows = {}  # qg -> [1, HPC*JPG, 128] recip-Z row tile
                    zr_pend = []  # [qg, zrcol, z_call_at_emit]
                    ob_q = []  # (tb, nb) out-proj blocks ready to emit
                    fs_state = {}  # tb -> current fs store-group tile
                    n_z = [0]
                    n_pv = [0]

                    def norm_step(pool, tag):
                        nqg, h, _ = norm_q.pop(0)
                        qs0 = nqg * TB
                        zbp = pool.tile([128, TB], F32, tag=tag, name="zbp")
                        nc.tensor.matmul(
                            zbp,
                            lhsT=ones_row,
                            rhs=zr_rows[nqg][0:1, h * JPG : (h + 1) * JPG, :],
                            start=True,
                            stop=True,
                        )
                        nc.vector.tensor_mul(
                            oT[:, h, qs0 : qs0 + TB],
                            oT[:, h, qs0 : qs0 + TB],
                            zbp,
                        )
                        if h == HPC - 1:  # group fully normalized
                            ob_q.extend(
                                (nqg, nb) for nb in range(D // 128)
                            )

                    def emit_block(btb, nb, pool, tag):
                        """One 128-column out-projection block: 2 matmuls
                        into a borrowed PSUM slot, cast into the current
                        4-block store group, DMA when the group completes."""
                        tbs = slice(btb * TB, (btb + 1) * TB)
                        nbs = slice(nb * 128, (nb + 1) * 128)
                        fo = pool.tile([128, TB], F32, tag=tag, name="fo")
                        for m in range(HPC):
                            nc.tensor.matmul(
                                fo,
                                lhsT=wo_sb[:, m, nbs],
                                rhs=oT[:, m, tbs],
                                start=(m == 0),
                                stop=(m == HPC - 1),
                            )
                        if b == B - 1 and btb == NTB_B - 1 and nb >= 12:
                            # shrink the final stores: the last DMA's
                            # latency is the kernel's tail
                            grp = 2 if nb < 14 else 1
                        else:
                            grp = 4
                        if nb % grp == 0:
                            fs_state[btb] = fsb.tile(
                                [128, 4, TB], F16, tag="fs", name="fs"
                            )
                        fs = fs_state[btb]
                        # 9/16 of casts on Act (DVE also carries the norm
                        # muls and zr copies); the kernel's final two casts
                        # go to Act, whose queue is empty at the end
                        on_act = nb % 2 == 0 or nb % 16 == 7 or grp == 1
                        if on_act:
                            nc.scalar.activation(fs[:, nb % grp, :], fo, Copy)
                        else:
                            nc.vector.tensor_copy(fs[:, nb % grp, :], fo)
                        if nb % grp == grp - 1:
                            # stores alternate between the Pool and SP
                            # queues (either alone serializes ~1.6us per
                            # store, which would gate the batch tail); the
                            # kernel's very last store goes to the Act
                            # queue, which is empty at the end
                            last = (
                                b == B - 1
                                and btb == NTB_B - 1
                                and nb == D // 128 - 1
                            )
                            if last:
                                deng = nc.scalar
                            elif (btb * 16 + nb) % 8 < 4:
                                deng = nc.gpsimd
                            else:
                                deng = nc.sync
                            deng.dma_start(
                                out=out_v[
                                    :,
                                    nb - grp + 1 : nb + 1,
                                    t0 + btb * TB : t0 + (btb + 1) * TB,
                                ],
                                in_=fs[:, 0:grp, :],
                            )

                    def process_zr():
                        """Late half of the Z path: PE-transpose the recip-Z
                        columns to [8,128] (riding the out-proj PSUM slot),
                        copy PSUM->SBUF on Act, and DMA the 8 rows into a
                        single [1, 8*128] row for the broadcast matmul."""
                        zqg, zrcol, _ = zr_pend.pop(0)
                        zrT = fop.tile(
                            [HPC * JPG, 128], BF16, tag="foA", name="zrT"
                        )
                        nc.tensor.transpose(zrT, zrcol, ident)
                        zrS = rp.tile([HPC * JPG, 128], BF16, tag="zs8")
                        # DVE, not Act: at the batch tail Act still has the
                        # trailing exps queued, which would delay the DMA
                        nc.vector.tensor_copy(zrS, zrT)
                        zrow_t = rp.tile(
                            [1, HPC * JPG, 128], BF16, tag="zrow"
                        )
                        # idle Pool queue: the SP queue carries x loads and
                        # output stores, which would delay this tiny DMA
                        nc.gpsimd.dma_start(out=zrow_t, in_=zrS)
                        zr_rows[zqg] = zrow_t
                        for h in range(HPC):
                            norm_q.append([zqg, h, n_pv[0] + 5 + 2 * h])

                    # (qg, j) chunk schedule, flattened; (0,0) was
                    # pre-warmed inside the projection scope
                    sched = [
                        (qg, j)
                        for qg in range(NTB_B)
                        for j in range(JPG * (qg + 1))
                    ][2:]
                    po = {}
                    zcol = {}
                    pend = warm  # [qg, j, (pt2, off), z_done]

                    def z_step(pqg, pj, pts):
                        """Z partial sums (trail 2) as transposed matmuls:
                        one [128,1] PSUM column per (head, 128-wide tq sub),
                        accumulated over key chunks. Each column's stop fires
                        on its last causal chunk; the recip covers all 8
                        columns in one DVE op on the group's last chunk."""
                        n_z[0] += 1
                        if zr_pend and n_z[0] - zr_pend[0][2] >= 2:
                            process_zr()
                        base = JPG * pqg
                        if pqg not in zcol:
                            zcol[pqg] = zpp.tile(
                                [128, HPC * JPG], F32, tag="zc", name="zc"
                            )
                        zc = zcol[pqg]
                        pt2, offp = pts
                        sub0 = max(0, pj - base)
                        # all 8 columns share ONE accumulation group (PSUM
                        # groups are tracked per 2KB zero region = the whole
                        # bank): start only on the group's first matmul --
                        # its start bit marks the bank pending-zero, so the
                        # other columns' first writes see zeros -- and stop
                        # only on its very last
                        for h in range(HPC):
                            for s in range(sub0, JPG):
                                nc.tensor.matmul(
                                    zc[:, h * JPG + s : h * JPG + s + 1],
                                    lhsT=pt2[:, h, s * 128 : (s + 1) * 128],
                                    rhs=ones_col,
                                    start=(pj == 0 and h == 0 and s == 0),
                                    stop=(
                                        pj == JPG * (pqg + 1) - 1
                                        and h == HPC - 1
                                        and s == JPG - 1
                                    ),
                                )
                        if pj == JPG * (pqg + 1) - 1:
                            zrcol = rp.tile(
                                [128, HPC * JPG], BF16, tag="zrc"
                            )
                            with nc.allow_low_precision(
                                reason="bf16 1/Z: 0.4% rel, in tolerance"
                            ):
                                nc.vector.reciprocal(zrcol, zc)
                            zr_pend.append([pqg, zrcol, n_z[0]])

                    def pv_step(pqg, pj, pts):
                        """PV matmuls (trail 3); one paced 1/Z broadcast may
                        ride along per call once its DMA has had >= 5 chunks
                        to land. Returns True if a broadcast was emitted."""
                        n_pv[0] += 1
                        popped = False
                        if norm_q and norm_q[0][2] <= n_pv[0]:
                            norm_step(fop, "foA")
                            popped = True
                        pjmax = JPG * (pqg + 1)
                        if (pqg, 0) not in po:
                            for h in range(HPC):
                                po[pqg, h] = pvp.tile(
                                    [128, TB], F32, tag=f"po{h}", name=f"po{h}"
                                )
                        pt2, offp = pts
                        for h in range(HPC):
                            nc.tensor.matmul(
                                po[pqg, h][:, offp:],
                                lhsT=vN[:, pj, h * HD : (h + 1) * HD],
                                rhs=pt2[:, h, offp:],
                                start=(pj == 0),
                                stop=(pj == pjmax - 1),
                            )
                        if pj == pjmax - 1:
                            qs0 = pqg * TB
                            for h in range(HPC):
                                # split across Act/DVE: the next group's po
                                # bank reuse waits on these
                                if h == 0:
                                    nc.scalar.activation(
                                        oT[:, h, qs0 : qs0 + TB],
                                        po[pqg, h],
                                        Copy,
                                    )
                                else:
                                    nc.vector.tensor_copy(
                                        oT[:, h, qs0 : qs0 + TB], po[pqg, h]
                                    )
                        return popped

                    # deferred V chunks: tb2's early (its vN rows are needed
                    # by group 2's PV), tb3's late; each takes the first
                    # conflict-free foA slot at/after its target chunk
                    defer_v = [(NTB_B - 2, s, (0, 1, 7, 11)[s]) for s in range(JPG)]
                    defer_v += [(1, 2, 4), (1, 3, 6)]
                    defer_v += [(NTB_B - 1, s, (15, 19, 23, 27)[s]) for s in range(JPG)]
                    defer_v.sort(key=lambda e: e[2])

                    def emit_v(vtb, s):
                        jv = vtb * JPG + s
                        xt_v_src = x_tiles[(b, vtb)]
                        pv = fop.tile(
                            [128, M_PC], F32, tag="foA", name="ps_vd"
                        )
                        for mc in range(NMC):
                            nc.tensor.matmul(
                                pv,
                                lhsT=xt_v_src[:, mc, s * 128 : (s + 1) * 128],
                                rhs=wv_sb[:, mc, :],
                                start=(mc == 0),
                                stop=(mc == NMC - 1),
                            )
                        nc.vector.tensor_copy(vN[:, jv, :], pv)

                    for n_ch, (qg, j) in enumerate(sched):
                        qs0 = qg * TB
                        off = max(0, (j - JPG * qg) * 128)
                        ks = slice(j * 128, (j + 1) * 128)
                        st2 = stp.tile([128, HPC, TB], F32, tag="st2")
                        for h in range(HPC):
                            nc.tensor.matmul(
                                st2[:, h, off:],
                                lhsT=kT[:, h, ks],
                                rhs=qT[:, h, qs0 + off : qs0 + TB],
                                start=True,
                                stop=True,
                            )
                        # exp first (it's on the PE's st2-ring critical
                        # path), then zero the diagonal block's masked wedge
                        # on the SBUF side where the 2-chunk Z/PV trail
                        # hides the DVE latency
                        pt2 = asb.tile([128, HPC, TB], BF16, tag="pt2")
                        nc.scalar.activation(
                            pt2[:, :, off:], st2[:, :, off:], Exp, scale=SCALE
                        )
                        pend.append([qg, j, (pt2, off), False])
                        popped = False
                        if len(pend) >= 5:
                            it = pend.pop(0)
                            popped = pv_step(it[0], it[1], it[2])
                        # masked-wedge zeroing emitted after pv_step: the
                        # group-end oT copies get DVE queue priority (the
                        # wedge isn't read until the Z step 2 chunks out)
                        if j >= JPG * qg:
                            for h in range(HPC):
                                nc.vector.tensor_mul(
                                    pt2[:, h, off : off + 128],
                                    pt2[:, h, off : off + 128],
                                    tri01,
                                )
                        if (
                            defer_v
                            and n_ch >= defer_v[0][2]
                            and not popped
                            and not zr_pend
                        ):
                            vtb, vs, _ = defer_v.pop(0)
                            emit_v(vtb, vs)
                        if b == 0 and n_ch == 10:
                            # b1's x loads: emitted once b0's deferred V
                            # matmuls (the last xt readers) are in the stream
                            for ltb in range(NTB_B):
                                load_x(1, ltb)
                        if len(pend) >= 3:
                            it = pend[-3]
                            z_step(it[0], it[1], it[2])
                            it[3] = True
                    for it in pend:
                        if not it[3]:
                            z_step(it[0], it[1], it[2])
                    while pend:
                        it = pend.pop(0)
                        pv_step(it[0], it[1], it[2])
                    # batch tail: all remaining out-proj blocks, rotating
                    # over five idle PSUM slots so each block's WAR lands
                    # on a cast several blocks back. A few blocks lead the
                    # last group's Z transpose (hiding the recip's DVE
                    # queue), and more cover its zr DMA before the norms.
                    rot = [
                        (stp, "st2"),
                        (pvp, "po0"),
                        (pvp, "po1"),
                        (fop, "foA"),
                        (zpp, "zc"),
                    ]
                    r = [0]

                    def tail_slot():
                        p, tg = rot[r[0] % len(rot)]
                        r[0] += 1
                        return p, tg

                    k = 0
                    while ob_q and k < 3:
                        emit_block(*ob_q.pop(0), *tail_slot())
                        k += 1
                    while zr_pend:
                        process_zr()
                    _mark(nc, f"b{b}_outproj")
                    first = True
                    while ob_q or norm_q:
                        k = 0
                        while ob_q and k < (8 if first else 4):
                            emit_block(*ob_q.pop(0), *tail_slot())
                            k += 1
                        first = False
                        if norm_q:
                            norm_step(*tail_slot())
    _legalize_waits(nc)
    return nc


_NC_CACHE = None


def _get_program():
    global _NC_CACHE
    if _NC_CACHE is None:
        _NC_CACHE = build_program()
    return _NC_CACHE


def _rope_tables():
    inv_freq = 1.0 / (ROPE_THETA ** (np.arange(0, HD, 2, dtype=np.float32) / HD))
    freqs = np.arange(T, dtype=np.float32)[:, None] * inv_freq[None, :]  # (T, 64)
    emb = np.concatenate([freqs, freqs], axis=-1)                        # (T, 128)
    cosT = np.ascontiguousarray(np.cos(emb).T).astype(BF16_NP)           # [128, T]
    sinT = np.sin(emb).T.astype(np.float32)
    sinhT = np.ascontiguousarray(
        np.concatenate([-sinT[: HD // 2], sinT[HD // 2 :]], axis=0)
    ).astype(BF16_NP)
    return cosT, sinhT


def kernel(x, Wq, Wk, Wv, Wo, **run_kwargs):
    x = np.asarray(x, dtype=np.float32)
    Wq = np.asarray(Wq, dtype=np.float32)
    Wk = np.asarray(Wk, dtype=np.float32)
    Wv = np.asarray(Wv, dtype=np.float32)
    Wo = np.asarray(Wo, dtype=np.float32)

    nc = _get_program()
    cosT, sinhT = _rope_tables()
    xT = np.ascontiguousarray(x.reshape(BT, D).T).astype(BF16_NP)  # [D, BT]
    # S^T[tk, tq] causal mask for the diagonal block: keep tq(col) >= tk(row)
    r = np.arange(128)
    triM = (r[None, :] >= r[:, None]).astype(BF16_NP)
    identM = np.eye(128, dtype=BF16_NP)

    in_maps = []
    for c in range(NCORES):
        sl = slice(c * M_PC, (c + 1) * M_PC)
        in_maps.append(
            {
                "xT": xT,
                "triT": triM,
                "identT": identM,
                "wqkT": np.ascontiguousarray(
                    np.concatenate([Wq[sl, :].T, Wk[sl, :].T], axis=1)
                ).astype(BF16_NP),
                "wvT": np.ascontiguousarray(Wv[sl, :].T).astype(BF16_NP),
                "woT": np.ascontiguousarray(Wo[:, sl].T).astype(BF16_NP),
                "cosT": cosT,
                "sinhT": sinhT,
            }
        )

    res = run_bass_kernel_spmd(nc, in_maps, list(range(NCORES)), **run_kwargs)
    acc = np.zeros((D, BT), dtype=np.float32)
    for c in range(NCORES):
        acc += res.results[c]["partialT"].astype(np.float32)
    out = np.ascontiguousarray(acc.T).reshape(B, T, D)
    if run_kwargs:
        return out, res
    return out



# revision 80
# speedup vs baseline: 1.0906x; 1.0069x over previous
"""Multi-head self-attention (B=2, T=2048, D=2048, H=16, RoPE, causal)
as a Bass/Tile kernel running SPMD on 8 trn2 NeuronCores.

Sharding: tensor-parallel over heads (2 heads per core). Each core
computes its heads' Q/K/V projections, RoPE, causal attention, and a
partial out-projection over its 256 feature columns; the host sums the
8 partial outputs (all-reduce equivalent).

Dataflow (per core, per batch):
  - x streamed per 512-wide t-block ([128, 16, 512] SBUF tiles, 4 tags);
    the first block's DMA is interleaved per-contraction-chunk with the
    weight loads so the PE starts ~2us in.
  - Q/K projections in "T-layout" (feature dim on partitions, time on
    free); RoPE rotate-half via two SBUF->SBUF partition-swap DMAs
    (sign folded into the sin table), all-bf16 combine on DVE (2x mode).
  - V projected in natural layout ([tk, d]): lhsT = x chunk, rhs = Wv
    slice -- no PE transposes. Only tb0 (+half of tb1) run in the
    projection phase; the rest defer into the attention window, where
    the PE would otherwise idle behind the exp stream.
  - scores computed transposed: S^T[tk, tq] per (key-chunk, q-group),
    narrowed to the block-causal minimum; both heads' tiles share one
    2-bank PSUM tile so a single exp covers them (Act is the scarce
    engine in this window). The causal wedge is zeroed AFTER the exp
    by a 0/1-triangle multiply on DVE -- off the exp's critical path.
  - softmax denominators as [128,1] PSUM columns via transposed
    matmuls (exp chunk stationary, ones moving): ~1 PE row per chunk
    instead of up-to-512, all 8 columns one accumulation group. The
    columns return to row layout off the PE: DVE reciprocal -> PE
    transpose (128 rows) -> Act copy -> 8-descriptor DMA on the idle
    Pool queue; the ones-row broadcast matmul + in-place DVE multiply
    trail by >=5 PV chunks so the DMA latency never stalls the PE.
  - the out-projection runs as a per-batch tail of 128-column blocks
    rotating over five drained PSUM slots, its casts split Act/DVE and
    stores alternating the Pool/SP queues; the last q-group's
    normalizations pop between blocks once their Z row has landed.
  - host sums the 8 cores' f16 partial outputs (all-reduce equivalent).
"""

import sys

sys.path.insert(0, "/opt/trn_rl_repo")

import ml_dtypes
import numpy as np

import concourse.bass as bass
import concourse.mybir as mybir
import concourse.tile as tile
from concourse.bass_utils import run_bass_kernel_spmd


def _legalize_waits(nc):
    """Walrus codegen rejects >2 sync waits on DMA/matmul/nop-class
    instructions, and Tile's pool-recycle waits bypass its own elision.
    Spill excess waits (>1) onto freshly inserted same-engine NoOps
    placed immediately before the offending instruction (sound w.r.t.
    per-engine program order)."""
    spill_id = [0]
    for bb in nc.m.functions[0].blocks:
        new_insts = []
        for inst in bb.instructions:
            si = getattr(inst, "sync_info", None)
            if si is None or not si.on_wait:
                new_insts.append(inst)
                continue
            eng = getattr(inst, "engine", None)
            kept = list(si.on_wait)
            if len(kept) > 1 and eng is not None:
                excess, kept = kept[:-1], kept[-1:]
                for w in excess:
                    spill_id[0] += 1
                    nop = mybir.InstNoOp(
                        name=f"I-wspill-{spill_id[0]}",
                        ins=[],
                        outs=[],
                        engine=eng,
                    )
                    nop.sync_info = mybir.SyncInfo(on_wait=[w], on_update=[])
                    new_insts.append(nop)
            if len(kept) != len(si.on_wait):
                si.on_wait[:] = kept
            new_insts.append(inst)
        if len(new_insts) != len(bb.instructions):
            bb.instructions[:] = new_insts


_PHASE_MARKS = []  # (phase_label, last_inst_index_before_phase) - profiling aid


def _mark(nc, label):
    n = -1
    for fn in nc.m.functions:
        for bb in fn.blocks:
            for ins in bb.instructions:
                if ins.name.startswith("I-"):
                    try:
                        n = max(n, int(ins.name[2:]))
                    except ValueError:
                        pass
    _PHASE_MARKS.append((label, n))


B, T, D, H, HD = 2, 2048, 2048, 16, 128
NCORES = 8
HPC = H // NCORES            # heads per core = 2
M_PC = HPC * HD              # per-core feature slice = 256
BT = B * T                   # 4096
SCALE = HD ** -0.5
ROPE_THETA = 10000.0

F32 = mybir.dt.float32
F16 = mybir.dt.float16
BF16 = mybir.dt.bfloat16
BF16_NP = ml_dtypes.bfloat16

TB = 512                     # t-block for projections / q-groups
NTB_B = T // TB              # 4 t-blocks per batch
NMC = D // 128               # 16 contraction chunks
NKC = T // 128               # 16 key chunks per batch
JPG = TB // 128              # key chunks per q-group width = 4

Copy = mybir.ActivationFunctionType.Copy
Exp = mybir.ActivationFunctionType.Exp


def build_program():
    nc = bass.Bass()

    xT_d = nc.declare_dram_parameter("xT", [D, BT], BF16, isOutput=False)
    tri_d = nc.declare_dram_parameter("triT", [128, 128], BF16, isOutput=False)
    # wq and wk concatenated so one DMA covers both (halves SP-seq time
    # on the critical startup path)
    wqk_d = nc.declare_dram_parameter(
        "wqkT", [D, 2 * M_PC], BF16, isOutput=False
    )
    wv_d = nc.declare_dram_parameter("wvT", [D, M_PC], BF16, isOutput=False)
    wo_d = nc.declare_dram_parameter("woT", [M_PC, D], BF16, isOutput=False)
    ident_d = nc.declare_dram_parameter("identT", [128, 128], BF16, isOutput=False)
    cos_d = nc.declare_dram_parameter("cosT", [HD, T], BF16, isOutput=False)
    sinh_d = nc.declare_dram_parameter("sinhT", [HD, T], BF16, isOutput=False)
    out_d = nc.declare_dram_parameter("partialT", [D, BT], F16, isOutput=True)

    xT_v = xT_d.rearrange("(c p) t -> p c t", p=128)      # [128, 16, BT]
    wqk_v = wqk_d.rearrange("(c p) n -> p c n", p=128)    # [128, 16, 512]
    wv_v = wv_d.rearrange("(c p) n -> p c n", p=128)
    wo_v = wo_d.rearrange("(c p) n -> p c n", p=128)      # [128, 2, 2048]
    out_v = out_d.rearrange("(c p) t -> p c t", p=128)    # [128, 16, BT]

    with tile.TileContext(nc) as tc:
        with (
            tc.tile_pool(name="wpool", bufs=1) as wpool,
            tc.tile_pool(name="xp", bufs=1) as xp,
            tc.tile_pool(name="big", bufs=1) as big,
            tc.tile_pool(name="rp", bufs=2) as rp,
            tc.tile_pool(name="attn_sb", bufs=8) as asb,
            tc.tile_pool(name="fs_sb", bufs=6) as fsb,
        ):
            # ---- weights + first x block, interleaved in graduated mc
            # groups (fast pipeline fill, then few big SP-cheap DMAs) ----
            wqk_sb = wpool.tile([128, NMC, 2 * M_PC], BF16, tag="wqk")
            wv_sb = wpool.tile([128, NMC, M_PC], BF16, tag="wv")
            x_tiles = {}
            xt0 = xp.tile([128, NMC, TB], BF16, tag="x0", name="x_b0_t0")
            x_tiles[(0, 0)] = xt0
            for lo, hi in ((0, 1), (1, 2), (2, 3), (3, 4), (4, 6), (6, 8),
                           (8, 10), (10, 12), (12, 14), (14, 16)):
                nc.sync.dma_start(
                    out=wqk_sb[:, lo:hi, :], in_=wqk_v[:, lo:hi, :]
                )
                # first x chunk rides the idle DVE queue, in parallel with
                # SP's weight DMA, to cut the cold-start latency
                eng = nc.scalar if lo == 0 else nc.sync
                eng.dma_start(
                    out=xt0[:, lo:hi, :], in_=xT_v[:, lo:hi, 0:TB]
                )

            cos_sb = wpool.tile([128, T], BF16, tag="cos")
            sinh_sb = wpool.tile([128, T], BF16, tag="sinh")
            nc.sync.dma_start(out=cos_sb[:, 0:TB], in_=cos_d[:, 0:TB])
            nc.sync.dma_start(out=sinh_sb[:, 0:TB], in_=sinh_d[:, 0:TB])

            def load_x(b, tb):
                t = xp.tile(
                    [128, NMC, TB], BF16, tag=f"x{tb}", name=f"x_b{b}_t{tb}"
                )
                x_tiles[(b, tb)] = t
                lo = b * T + tb * TB
                for m0 in range(0, NMC, 4):
                    nc.sync.dma_start(
                        out=t[:, m0 : m0 + 4, :],
                        in_=xT_v[:, m0 : m0 + 4, lo : lo + TB],
                    )

            # wv rides alongside tb0's V matmuls; x block 1 follows
            for m0 in range(0, NMC, 4):
                nc.sync.dma_start(
                    out=wv_sb[:, m0 : m0 + 4, :], in_=wv_v[:, m0 : m0 + 4, :]
                )
            load_x(0, 1)
            nc.sync.dma_start(out=cos_sb[:, TB:], in_=cos_d[:, TB:])
            nc.sync.dma_start(out=sinh_sb[:, TB:], in_=sinh_d[:, TB:])
            tri01 = wpool.tile([128, 128], BF16, tag="tri01")
            nc.sync.dma_start(out=tri01, in_=tri_d[:, :])
            ident = wpool.tile([128, 128], BF16, tag="ident")
            nc.sync.dma_start(out=ident, in_=ident_d[:, :])
            ones_col = wpool.tile([128, 1], BF16, tag="ones_c")
            nc.vector.memset(ones_col, 1.0)
            ones_row = wpool.tile([1, 128], BF16, tag="ones_r")
            nc.vector.memset(ones_row, 1.0)

            for tb in range(2, NTB_B):
                load_x(0, tb)

            wo_sb = wpool.tile([128, HPC, D], BF16, tag="wo")
            nc.sync.dma_start(out=wo_sb, in_=wo_v)

            for b in range(B):
                t0 = b * T  # global t offset of this batch
                _mark(nc, f"b{b}_proj")

                # persistent per-batch tensors (slots reused across b)
                qT = big.tile([128, HPC, T], BF16, tag="qT")   # [hd, h, t]
                kT = big.tile([128, HPC, T], BF16, tag="kT")
                vN = big.tile([128, NKC, M_PC], BF16, tag="vN")  # [tk, j, n]
                oT = big.tile([128, HPC, T], BF16, tag="oT")   # attn out

                # ---------------- projections + RoPE ----------------
                warm = []  # pre-emitted attention chunks
                with tc.tile_pool(name="qk_ps", bufs=1, space="PSUM") as qkp, \
                     tc.tile_pool(name="v_ps", bufs=1, space="PSUM") as vps:
                    for tb in range(NTB_B):
                        xt = x_tiles[(b, tb)]
                        ts_l = slice(tb * TB, (tb + 1) * TB)   # in-batch
                        ps = {}
                        for h in range(HPC):
                            for nm in ("q", "k"):
                                ps[nm, h] = qkp.tile(
                                    [128, TB], F32, tag=f"{nm}{h}",
                                    name=f"ps_{nm}{h}",
                                )
                        for mc in range(NMC):
                            for h in range(HPC):
                                for ni, nm in ((0, "q"), (1, "k")):
                                    hs = slice(
                                        ni * M_PC + h * HD,
                                        ni * M_PC + (h + 1) * HD,
                                    )
                                    nc.tensor.matmul(
                                        ps[nm, h],
                                        lhsT=wqk_sb[:, mc, hs],
                                        rhs=xt[:, mc, :],
                                        start=(mc == 0),
                                        stop=(mc == NMC - 1),
                                    )
                        # raw bf16 casts early on Act (deps ready now)
                        raws = {}
                        for nm in ("q", "k"):
                            for h in range(HPC):
                                raw = rp.tile(
                                    [128, TB], BF16, tag=f"raw{nm}{h}"
                                )
                                nc.scalar.activation(raw, ps[nm, h], Copy)
                                raws[nm, h] = raw
                        # RoPE before V: the qk/sw PSUM banks drain while the
                        # PE streams V matmuls, so the next phase's bank-WAR
                        # waits resolve before the PE gets there
                        for nm, dest in (("q", qT), ("k", kT)):
                            for h in range(HPC):
                                raw = raws[nm, h]
                                swb = rp.tile([128, TB], BF16, tag="swb")
                                nc.sync.dma_start(
                                    out=swb[0:64, :], in_=raw[64:128, :]
                                )
                                nc.sync.dma_start(
                                    out=swb[64:128, :], in_=raw[0:64, :]
                                )
                                t2 = rp.tile([128, TB], BF16, tag="t2")
                                nc.vector.tensor_mul(t2, swb, sinh_sb[:, ts_l])
                                t1 = rp.tile([128, TB], BF16, tag="t1")
                                nc.vector.tensor_mul(t1, raw, cos_sb[:, ts_l])
                                nc.vector.tensor_add(dest[:, h, ts_l], t1, t2)
                        # V directly in natural layout: per 128-wide tk chunk
                        def v_chunk(s):
                            j = tb * JPG + s
                            pv = vps.tile(
                                [128, M_PC], F32, tag=f"v{s % 2}",
                                name=f"ps_v{s % 2}",
                            )
                            for mc in range(NMC):
                                nc.tensor.matmul(
                                    pv,
                                    lhsT=xt[:, mc, s * 128 : (s + 1) * 128],
                                    rhs=wv_sb[:, mc, :],
                                    start=(mc == 0),
                                    stop=(mc == NMC - 1),
                                )
                            nc.scalar.activation(vN[:, j, :], pv, Copy)

                        for s in range(JPG):
                            # tb1's tail / tb2 / tb3 V chunks are deferred
                            # into the attention chunks (the window is
                            # Act-bound there; these give the PE ~1.7us
                            # each and move their Act/DVE load out of the
                            # backlog)
                            if tb == 0 or (tb == 1 and s < 2):
                                v_chunk(s)
                            # between tb3's V chunks, warm up attention: the
                            # (qg0, j0) score pair + exp run in drained qk
                            # PSUM slots, their latency hidden by V matmuls,
                            # so the attention loop starts with a full
                            # pipeline instead of a fill bubble
                            if tb == 2 and s in (1, 2):
                                # chunk (0,0) after V s1 in the q-tag slots,
                                # chunk (0,1) after V s2 in the k-tag slots:
                                # the attention loop starts two chunks deep,
                                # matching the Z trail exactly
                                wj = 0 if s == 1 else 1
                                woff = 0 if wj == 0 else 128
                                wtag = "q" if s == 1 else "k"
                                wpt2 = asb.tile(
                                    [128, HPC, TB], BF16, tag="pt2",
                                    name="warm_pt",
                                )
                                for h in range(HPC):
                                    wst = qkp.tile(
                                        [128, TB], F32,
                                        tag=f"{wtag}{h}", name="warm_st",
                                    )
                                    nc.tensor.matmul(
                                        wst[:, woff:],
                                        lhsT=kT[:, h, wj * 128 : (wj + 1) * 128],
                                        rhs=qT[:, h, woff:TB],
                                        start=True,
                                        stop=True,
                                    )
                                    nc.scalar.activation(
                                        wpt2[:, h, woff:], wst[:, woff:],
                                        Exp, scale=SCALE,
                                    )
                                    nc.vector.tensor_mul(
                                        wpt2[:, h, woff : woff + 128],
                                        wpt2[:, h, woff : woff + 128],
                                        tri01,
                                    )
                                warm.append([0, wj, (wpt2, woff), False])



                _mark(nc, f"b{b}_attn")
                # ---------------- attention + out-projection ----------------
                # Both heads' score tiles share one 2-bank PSUM tile so a
                # single exp call covers them (Act is the scarce engine in
                # this window). Z/PV trail the score/exp stream by 2/3
                # chunks, software-pipelined ACROSS q-groups. Z is
                # accumulated as [128,1] columns via transposed matmuls
                # (exp chunk stationary, ones moving) -- ~1 PE row per chunk
                # instead of up-to-512 -- and routed back to row layout off
                # the PE (recip -> PE transpose -> Act copy -> 8-descriptor
                # DMA). The out-projection interleaves with the chunk
                # stream, one 128-column block per chunk once its q-group is
                # normalized, keeping the PE fed while Act drains exps; the
                # leftovers plus the last group's norms form the batch tail
                # on the score-tile ring.
                with tc.tile_pool(name="st_ps", bufs=2, space="PSUM") as stp, \
                     tc.tile_pool(name="pv_ps", bufs=1, space="PSUM") as pvp, \
                     tc.tile_pool(name="z_ps", bufs=1, space="PSUM") as zpp, \
                     tc.tile_pool(name="fo_ps", bufs=1, space="PSUM") as fop:
                    norm_q = []  # pending [qg, h, ready_at_pv_call]
                    zr_rows = {}  # qg -> [1, HPC*JPG, 128] recip-Z row tile
                    zr_pend = []  # [qg, zrcol, z_call_at_emit]
                    ob_q = []  # (tb, nb) out-proj blocks ready to emit
                    fs_state = {}  # tb -> current fs store-group tile
                    n_z = [0]
                    n_pv = [0]

                    def norm_step(pool, tag):
                        nqg, h, _ = norm_q.pop(0)
                        qs0 = nqg * TB
                        zbp = pool.tile([128, TB], F32, tag=tag, name="zbp")
                        nc.tensor.matmul(
                            zbp,
                            lhsT=ones_row,
                            rhs=zr_rows[nqg][0:1, h * JPG : (h + 1) * JPG, :],
                            start=True,
                            stop=True,
                        )
                        nc.vector.tensor_mul(
                            oT[:, h, qs0 : qs0 + TB],
                            oT[:, h, qs0 : qs0 + TB],
                            zbp,
                        )
                        if h == HPC - 1:  # group fully normalized
                            ob_q.extend(
                                (nqg, nb) for nb in range(D // 128)
                            )

                    def emit_block(btb, nb, pool, tag):
                        """One 128-column out-projection block: 2 matmuls
                        into a borrowed PSUM slot, cast into the current
                        4-block store group, DMA when the group completes."""
                        tbs = slice(btb * TB, (btb + 1) * TB)
                        nbs = slice(nb * 128, (nb + 1) * 128)
                        fo = pool.tile([128, TB], F32, tag=tag, name="fo")
                        for m in range(HPC):
                            nc.tensor.matmul(
                                fo,
                                lhsT=wo_sb[:, m, nbs],
                                rhs=oT[:, m, tbs],
                                start=(m == 0),
                                stop=(m == HPC - 1),
                            )
                        if b == B - 1 and btb == NTB_B - 1 and nb >= 12:
                            # shrink the final stores: the last DMA's
                            # latency is the kernel's tail
                            grp = 2 if nb < 14 else 1
                        else:
                            grp = 4
                        if nb % grp == 0:
                            fs_state[btb] = fsb.tile(
                                [128, 4, TB], F16, tag="fs", name="fs"
                            )
                        fs = fs_state[btb]
                        # 9/16 of casts on Act (DVE also carries the norm
                        # muls and zr copies); the kernel's final two casts
                        # go to Act, whose queue is empty at the end
                        on_act = nb % 2 == 0 or nb % 16 == 7 or grp == 1
                        if on_act:
                            nc.scalar.activation(fs[:, nb % grp, :], fo, Copy)
                        else:
                            nc.vector.tensor_copy(fs[:, nb % grp, :], fo)
                        if nb % grp == grp - 1:
                            # stores alternate between the Pool and SP
                            # queues (either alone serializes ~1.6us per
                            # store, which would gate the batch tail); the
                            # kernel's very last store goes to the Act
                            # queue, which is empty at the end
                            last = (
                                b == B - 1
                                and btb == NTB_B - 1
                                and nb == D // 128 - 1
                            )
                            if last:
                                deng = nc.scalar
                            elif (btb * 16 + nb) % 8 < 4:
                                deng = nc.gpsimd
                            else:
                                deng = nc.sync
                            deng.dma_start(
                                out=out_v[
                                    :,
                                    nb - grp + 1 : nb + 1,
                                    t0 + btb * TB : t0 + (btb + 1) * TB,
                                ],
                                in_=fs[:, 0:grp, :],
                            )

                    def process_zr():
                        """Late half of the Z path: PE-transpose the recip-Z
                        columns to [8,128] (riding the out-proj PSUM slot),
                        copy PSUM->SBUF on Act, and DMA the 8 rows into a
                        single [1, 8*128] row for the broadcast matmul."""
                        zqg, zrcol, _ = zr_pend.pop(0)
                        zrT = fop.tile(
                            [HPC * JPG, 128], BF16, tag="foA", name="zrT"
                        )
                        nc.tensor.transpose(zrT, zrcol, ident)
                        zrS = rp.tile([HPC * JPG, 128], BF16, tag="zs8")
                        # DVE, not Act: at the batch tail Act still has the
                        # trailing exps queued, which would delay the DMA
                        nc.vector.tensor_copy(zrS, zrT)
                        zrow_t = rp.tile(
                            [1, HPC * JPG, 128], BF16, tag="zrow"
                        )
                        # idle Pool queue: the SP queue carries x loads and
                        # output stores, which would delay this tiny DMA
                        nc.gpsimd.dma_start(out=zrow_t, in_=zrS)
                        zr_rows[zqg] = zrow_t
                        for h in range(HPC):
                            norm_q.append([zqg, h, n_pv[0] + 5 + 2 * h])

                    # (qg, j) chunk schedule, flattened; (0,0) was
                    # pre-warmed inside the projection scope
                    sched = [
                        (qg, j)
                        for qg in range(NTB_B)
                        for j in range(JPG * (qg + 1))
                    ][2:]
                    po = {}
                    zcol = {}
                    pend = warm  # [qg, j, (pt2, off), z_done]

                    def z_step(pqg, pj, pts):
                        """Z partial sums (trail 2) as transposed matmuls:
                        one [128,1] PSUM column per (head, 128-wide tq sub),
                        accumulated over key chunks. Each column's stop fires
                        on its last causal chunk; the recip covers all 8
                        columns in one DVE op on the group's last chunk."""
                        n_z[0] += 1
                        if zr_pend and n_z[0] - zr_pend[0][2] >= 3:
                            process_zr()
                        base = JPG * pqg
                        if pqg not in zcol:
                            zcol[pqg] = zpp.tile(
                                [128, HPC * JPG], F32, tag="zc", name="zc"
                            )
                        zc = zcol[pqg]
                        pt2, offp = pts
                        sub0 = max(0, pj - base)
                        # all 8 columns share ONE accumulation group (PSUM
                        # groups are tracked per 2KB zero region = the whole
                        # bank): start only on the group's first matmul --
                        # its start bit marks the bank pending-zero, so the
                        # other columns' first writes see zeros -- and stop
                        # only on its very last
                        for h in range(HPC):
                            for s in range(sub0, JPG):
                                nc.tensor.matmul(
                                    zc[:, h * JPG + s : h * JPG + s + 1],
                                    lhsT=pt2[:, h, s * 128 : (s + 1) * 128],
                                    rhs=ones_col,
                                    start=(pj == 0 and h == 0 and s == 0),
                                    stop=(
                                        pj == JPG * (pqg + 1) - 1
                                        and h == HPC - 1
                                        and s == JPG - 1
                                    ),
                                )
                        if pj == JPG * (pqg + 1) - 1:
                            zrcol = rp.tile(
                                [128, HPC * JPG], BF16, tag="zrc"
                            )
                            with nc.allow_low_precision(
                                reason="bf16 1/Z: 0.4% rel, in tolerance"
                            ):
                                nc.vector.reciprocal(zrcol, zc)
                            zr_pend.append([pqg, zrcol, n_z[0]])

                    def pv_step(pqg, pj, pts):
                        """PV matmuls (trail 3); one paced 1/Z broadcast may
                        ride along per call once its DMA has had >= 5 chunks
                        to land. Returns True if a broadcast was emitted."""
                        n_pv[0] += 1
                        popped = False
                        if norm_q and norm_q[0][2] <= n_pv[0]:
                            norm_step(fop, "foA")
                            popped = True
                        pjmax = JPG * (pqg + 1)
                        if (pqg, 0) not in po:
                            for h in range(HPC):
                                po[pqg, h] = pvp.tile(
                                    [128, TB], F32, tag=f"po{h}", name=f"po{h}"
                                )
                        pt2, offp = pts
                        last_c = pj == pjmax - 1
                        qs0 = pqg * TB
                        # on the group's last chunk, finish h1 first and
                        # emit its (DVE) oT copy immediately -- the next
                        # group's po1 bank reuse waits on it; h0's copy
                        # rides Act
                        for h in ((1, 0) if last_c else (0, 1)):
                            nc.tensor.matmul(
                                po[pqg, h][:, offp:],
                                lhsT=vN[:, pj, h * HD : (h + 1) * HD],
                                rhs=pt2[:, h, offp:],
                                start=(pj == 0),
                                stop=last_c,
                            )
                            if last_c:
                                if h == 0:
                                    nc.scalar.activation(
                                        oT[:, h, qs0 : qs0 + TB],
                                        po[pqg, h],
                                        Copy,
                                    )
                                else:
                                    nc.vector.tensor_copy(
                                        oT[:, h, qs0 : qs0 + TB], po[pqg, h]
                                    )
                        return popped

                    # deferred V chunks: tb2's early (its vN rows are needed
                    # by group 2's PV), tb3's late; each takes the first
                    # conflict-free foA slot at/after its target chunk
                    defer_v = [(NTB_B - 2, s, (0, 1, 5, 9)[s]) for s in range(JPG)]
                    defer_v += [(1, 2, 3), (1, 3, 7)]
                    defer_v += [(NTB_B - 1, s, (13, 17, 21, 25)[s]) for s in range(JPG)]
                    defer_v.sort(key=lambda e: e[2])

                    def emit_v(vtb, s):
                        jv = vtb * JPG + s
                        xt_v_src = x_tiles[(b, vtb)]
                        pv = fop.tile(
                            [128, M_PC], F32, tag="foA", name="ps_vd"
                        )
                        for mc in range(NMC):
                            nc.tensor.matmul(
                                pv,
                                lhsT=xt_v_src[:, mc, s * 128 : (s + 1) * 128],
                                rhs=wv_sb[:, mc, :],
                                start=(mc == 0),
                                stop=(mc == NMC - 1),
                            )
                        nc.vector.tensor_copy(vN[:, jv, :], pv)

                    for n_ch, (qg, j) in enumerate(sched):
                        qs0 = qg * TB
                        off = max(0, (j - JPG * qg) * 128)
                        ks = slice(j * 128, (j + 1) * 128)
                        st2 = stp.tile([128, HPC, TB], F32, tag="st2")
                        for h in range(HPC):
                            nc.tensor.matmul(
                                st2[:, h, off:],
                                lhsT=kT[:, h, ks],
                                rhs=qT[:, h, qs0 + off : qs0 + TB],
                                start=True,
                                stop=True,
                            )
                        # exp first (it's on the PE's st2-ring critical
                        # path), then zero the diagonal block's masked wedge
                        # on the SBUF side where the 2-chunk Z/PV trail
                        # hides the DVE latency
                        pt2 = asb.tile([128, HPC, TB], BF16, tag="pt2")
                        nc.scalar.activation(
                            pt2[:, :, off:], st2[:, :, off:], Exp, scale=SCALE
                        )
                        pend.append([qg, j, (pt2, off), False])
                        popped = False
                        if len(pend) >= 5:
                            it = pend.pop(0)
                            popped = pv_step(it[0], it[1], it[2])
                        # masked-wedge zeroing emitted after pv_step: the
                        # group-end oT copies get DVE queue priority (the
                        # wedge isn't read until the Z step 2 chunks out)
                        if j >= JPG * qg:
                            for h in range(HPC):
                                nc.vector.tensor_mul(
                                    pt2[:, h, off : off + 128],
                                    pt2[:, h, off : off + 128],
                                    tri01,
                                )
                        if (
                            defer_v
                            and n_ch >= defer_v[0][2]
                            and not popped
                            and not zr_pend
                        ):
                            vtb, vs, _ = defer_v.pop(0)
                            emit_v(vtb, vs)
                        if b == 0 and n_ch == 10:
                            # b1's x loads: emitted once b0's deferred V
                            # matmuls (the last xt readers) are in the stream
                            for ltb in range(NTB_B):
                                load_x(1, ltb)
                        if len(pend) >= 3:
                            it = pend[-3]
                            z_step(it[0], it[1], it[2])
                            it[3] = True
                    for it in pend:
                        if not it[3]:
                            z_step(it[0], it[1], it[2])
                    while pend:
                        it = pend.pop(0)
                        pv_step(it[0], it[1], it[2])
                    # batch tail: all remaining out-proj blocks, rotating
                    # over five idle PSUM slots so each block's WAR lands
                    # on a cast several blocks back. A few blocks lead the
                    # last group's Z transpose (hiding the recip's DVE
                    # queue), and more cover its zr DMA before the norms.
                    rot = [
                        (stp, "st2"),
                        (pvp, "po0"),
                        (pvp, "po1"),
                        (fop, "foA"),
                        (zpp, "zc"),
                    ]
                    r = [0]

                    def tail_slot():
                        p, tg = rot[r[0] % len(rot)]
                        r[0] += 1
                        return p, tg

                    k = 0
                    while ob_q and k < 3:
                        emit_block(*ob_q.pop(0), *tail_slot())
                        k += 1
                    while zr_pend:
                        process_zr()
                    _mark(nc, f"b{b}_outproj")
                    first = True
                    while ob_q or norm_q:
                        k = 0
                        while ob_q and k < (8 if first else 4):
                            emit_block(*ob_q.pop(0), *tail_slot())
                            k += 1
                        first = False
                        if norm_q:
                            norm_step(*tail_slot())
    _legalize_waits(nc)
    return nc


_NC_CACHE = None


def _get_program():
    global _NC_CACHE
    if _NC_CACHE is None:
        _NC_CACHE = build_program()
    return _NC_CACHE


def _rope_tables():
    inv_freq = 1.0 / (ROPE_THETA ** (np.arange(0, HD, 2, dtype=np.float32) / HD))
    freqs = np.arange(T, dtype=np.float32)[:, None] * inv_freq[None, :]  # (T, 64)
    emb = np.concatenate([freqs, freqs], axis=-1)                        # (T, 128)
    cosT = np.ascontiguousarray(np.cos(emb).T).astype(BF16_NP)           # [128, T]
    sinT = np.sin(emb).T.astype(np.float32)
    sinhT = np.ascontiguousarray(
        np.concatenate([-sinT[: HD // 2], sinT[HD // 2 :]], axis=0)
    ).astype(BF16_NP)
    return cosT, sinhT


def kernel(x, Wq, Wk, Wv, Wo, **run_kwargs):
    x = np.asarray(x, dtype=np.float32)
    Wq = np.asarray(Wq, dtype=np.float32)
    Wk = np.asarray(Wk, dtype=np.float32)
    Wv = np.asarray(Wv, dtype=np.float32)
    Wo = np.asarray(Wo, dtype=np.float32)

    nc = _get_program()
    cosT, sinhT = _rope_tables()
    xT = np.ascontiguousarray(x.reshape(BT, D).T).astype(BF16_NP)  # [D, BT]
    # S^T[tk, tq] causal mask for the diagonal block: keep tq(col) >= tk(row)
    r = np.arange(128)
    triM = (r[None, :] >= r[:, None]).astype(BF16_NP)
    identM = np.eye(128, dtype=BF16_NP)

    in_maps = []
    for c in range(NCORES):
        sl = slice(c * M_PC, (c + 1) * M_PC)
        in_maps.append(
            {
                "xT": xT,
                "triT": triM,
                "identT": identM,
                "wqkT": np.ascontiguousarray(
                    np.concatenate([Wq[sl, :].T, Wk[sl, :].T], axis=1)
                ).astype(BF16_NP),
                "wvT": np.ascontiguousarray(Wv[sl, :].T).astype(BF16_NP),
                "woT": np.ascontiguousarray(Wo[:, sl].T).astype(BF16_NP),
                "cosT": cosT,
                "sinhT": sinhT,
            }
        )

    res = run_bass_kernel_spmd(nc, in_maps, list(range(NCORES)), **run_kwargs)
    acc = np.zeros((D, BT), dtype=np.float32)
    for c in range(NCORES):
        acc += res.results[c]["partialT"].astype(np.float32)
    out = np.ascontiguousarray(acc.T).reshape(B, T, D)
    if run_kwargs:
        return out, res
    return out



# revision 91
# speedup vs baseline: 1.2576x; 1.1531x over previous
"""Multi-head self-attention (B=2, T=2048, D=2048, H=16, RoPE, causal)
as a Bass/Tile kernel running SPMD on 8 trn2 NeuronCores.

Sharding: tensor-parallel over heads (2 heads per core). Each core
computes its heads' Q/K/V projections, RoPE, causal attention, and a
partial out-projection over its 256 feature columns; the host sums the
8 partial outputs (all-reduce equivalent).

Dataflow (per core, per batch):
  - x streamed per 512-wide t-block ([128, 16, 512] SBUF tiles, 4 tags);
    the first block's DMA is interleaved per-contraction-chunk with the
    weight loads so the PE starts ~2us in.
  - Q/K projections in "T-layout" (feature dim on partitions, time on
    free); RoPE rotate-half via two SBUF->SBUF partition-swap DMAs
    (sign folded into the sin table), all-bf16 combine on DVE (2x mode).
  - V projected in natural layout ([tk, d]): lhsT = x chunk, rhs = Wv
    slice -- no PE transposes. Only tb0 (+half of tb1) run in the
    projection phase; the rest defer into the attention window, where
    the PE would otherwise idle behind the exp stream.
  - scores computed transposed: S^T[tk, tq] per (key-chunk, q-group),
    narrowed to the block-causal minimum; both heads' tiles share one
    2-bank PSUM tile so a single exp covers them (Act is the scarce
    engine in this window). The causal wedge is zeroed AFTER the exp
    by a 0/1-triangle multiply on DVE -- off the exp's critical path.
  - softmax denominators as [128,1] PSUM columns via transposed
    matmuls (exp chunk stationary, ones moving): ~1 PE row per chunk
    instead of up-to-512, all 8 columns one accumulation group. The
    columns return to row layout off the PE: DVE reciprocal -> PE
    transpose (128 rows) -> Act copy -> 8-descriptor DMA on the idle
    Pool queue; the ones-row broadcast matmul + in-place DVE multiply
    trail by >=5 PV chunks so the DMA latency never stalls the PE.
  - the out-projection runs as a per-batch tail of 128-column blocks
    rotating over five drained PSUM slots, its casts split Act/DVE and
    stores alternating the Pool/SP queues; the last q-group's
    normalizations pop between blocks once their Z row has landed.
  - host sums the 8 cores' f16 partial outputs (all-reduce equivalent).
"""

import sys

sys.path.insert(0, "/opt/trn_rl_repo")

import ml_dtypes
import numpy as np

import concourse.bass as bass
import concourse.mybir as mybir
import concourse.tile as tile
from concourse.bass_utils import run_bass_kernel_spmd


def _legalize_waits(nc):
    """Walrus codegen rejects >2 sync waits on DMA/matmul/nop-class
    instructions, and Tile's pool-recycle waits bypass its own elision.
    Spill excess waits (>1) onto freshly inserted same-engine NoOps
    placed immediately before the offending instruction (sound w.r.t.
    per-engine program order)."""
    spill_id = [0]
    for bb in nc.m.functions[0].blocks:
        new_insts = []
        for inst in bb.instructions:
            si = getattr(inst, "sync_info", None)
            if si is None or not si.on_wait:
                new_insts.append(inst)
                continue
            eng = getattr(inst, "engine", None)
            kept = list(si.on_wait)
            if len(kept) > 1 and eng is not None:
                excess, kept = kept[:-1], kept[-1:]
                for w in excess:
                    spill_id[0] += 1
                    nop = mybir.InstNoOp(
                        name=f"I-wspill-{spill_id[0]}",
                        ins=[],
                        outs=[],
                        engine=eng,
                    )
                    nop.sync_info = mybir.SyncInfo(on_wait=[w], on_update=[])
                    new_insts.append(nop)
            if len(kept) != len(si.on_wait):
                si.on_wait[:] = kept
            new_insts.append(inst)
        if len(new_insts) != len(bb.instructions):
            bb.instructions[:] = new_insts


_PHASE_MARKS = []  # (phase_label, last_inst_index_before_phase) - profiling aid


def _mark(nc, label):
    n = -1
    for fn in nc.m.functions:
        for bb in fn.blocks:
            for ins in bb.instructions:
                if ins.name.startswith("I-"):
                    try:
                        n = max(n, int(ins.name[2:]))
                    except ValueError:
                        pass
    _PHASE_MARKS.append((label, n))


B, T, D, H, HD = 2, 2048, 2048, 16, 128
NCORES = 8
HPC = H // NCORES            # heads per core = 2
M_PC = HPC * HD              # per-core feature slice = 256
BT = B * T                   # 4096
SCALE = HD ** -0.5
ROPE_THETA = 10000.0

F32 = mybir.dt.float32
F16 = mybir.dt.float16
BF16 = mybir.dt.bfloat16
FP8 = mybir.dt.float8e4
BF16_NP = ml_dtypes.bfloat16
FP8_NP = ml_dtypes.float8_e4m3fn
WSCALE = 64.0                # fp8 weight pre-scale (folded into exp / host sum)
SCALE_EXP = (HD ** -0.5) / (WSCALE * WSCALE)  # softmax scale, Wq/Wk pre-scale folded in

TB = 512                     # t-block for projections / q-groups
NTB_B = T // TB              # 4 t-blocks per batch
NMC = D // 128               # 16 contraction chunks
NKC = T // 128               # 16 key chunks per batch
JPG = TB // 128              # key chunks per q-group width = 4

Copy = mybir.ActivationFunctionType.Copy
Exp = mybir.ActivationFunctionType.Exp


def build_program():
    nc = bass.Bass()

    # x and the QKV weights ship as fp8 e4m3 value+residual pairs: the
    # projections run as three DoubleRow matmul streams
    # (x8*W8 + x8*dW8 + dx8*W8), 2 K-tiles per instruction at 0.5
    # cycles/row -- 0.75x the bf16 PE cost at ~bf16 accuracy. Weights
    # are pre-scaled by WSCALE into e4m3's dynamic range; the scale
    # cancels in softmax (exp scale) and the host sum (/WSCALE).
    x8_d = nc.declare_dram_parameter("xT8", [D, BT], FP8, isOutput=False)
    dx8_d = nc.declare_dram_parameter("dxT8", [D, BT], FP8, isOutput=False)
    tri_d = nc.declare_dram_parameter("triT", [128, 128], BF16, isOutput=False)
    # wq and wk concatenated so one DMA covers both (halves SP-seq time
    # on the critical startup path)
    wqk_d = nc.declare_dram_parameter(
        "wqk8T", [D, 2 * M_PC], FP8, isOutput=False
    )
    dwqk_d = nc.declare_dram_parameter(
        "dwqk8T", [D, 2 * M_PC], FP8, isOutput=False
    )
    wv_d = nc.declare_dram_parameter("wv8T", [D, M_PC], FP8, isOutput=False)
    dwv_d = nc.declare_dram_parameter("dwv8T", [D, M_PC], FP8, isOutput=False)
    wo_d = nc.declare_dram_parameter("woT", [M_PC, D], BF16, isOutput=False)
    ident_d = nc.declare_dram_parameter("identT", [128, 128], BF16, isOutput=False)
    cos_d = nc.declare_dram_parameter("cosT", [HD, T], BF16, isOutput=False)
    sinh_d = nc.declare_dram_parameter("sinhT", [HD, T], BF16, isOutput=False)
    out_d = nc.declare_dram_parameter("partialT", [D, BT], F16, isOutput=True)

    x8_v = x8_d.rearrange("(c p) t -> p c t", p=128)      # [128, 16, BT]
    dx8_v = dx8_d.rearrange("(c p) t -> p c t", p=128)
    wqk_v = wqk_d.rearrange("(c p) n -> p c n", p=128)    # [128, 16, 512]
    dwqk_v = dwqk_d.rearrange("(c p) n -> p c n", p=128)
    wv_v = wv_d.rearrange("(c p) n -> p c n", p=128)
    dwv_v = dwv_d.rearrange("(c p) n -> p c n", p=128)
    wo_v = wo_d.rearrange("(c p) n -> p c n", p=128)      # [128, 2, 2048]
    out_v = out_d.rearrange("(c p) t -> p c t", p=128)    # [128, 16, BT]

    with tile.TileContext(nc) as tc:
        with (
            tc.tile_pool(name="wpool", bufs=1) as wpool,
            tc.tile_pool(name="xp", bufs=1) as xp,
            tc.tile_pool(name="big", bufs=1) as big,
            tc.tile_pool(name="rp", bufs=2) as rp,
            tc.tile_pool(name="attn_sb", bufs=8) as asb,
            tc.tile_pool(name="fs_sb", bufs=6) as fsb,
        ):
            # ---- weights + first x block, interleaved in graduated mc
            # groups (fast pipeline fill, then few big SP-cheap DMAs) ----
            wqk_sb = wpool.tile([128, NMC, 2 * M_PC], FP8, tag="wqk")
            dwqk_sb = wpool.tile([128, NMC, 2 * M_PC], FP8, tag="dwqk")
            wv_sb = wpool.tile([128, NMC, M_PC], FP8, tag="wv")
            dwv_sb = wpool.tile([128, NMC, M_PC], FP8, tag="dwv")
            x_tiles = {}
            xt0 = xp.tile([128, NMC, TB], FP8, tag="x0", name="x8_b0_t0")
            dxt0 = xp.tile([128, NMC, TB], FP8, tag="dx0", name="dx8_b0_t0")
            x_tiles[(0, 0)] = (xt0, dxt0)
            # value streams (w8, x8) ride SP; residual streams (dw8, dx8)
            # ride the parallel Act queue -- halves the startup
            # serialization now that every tensor ships as a pair
            for lo, hi in ((0, 2), (2, 4), (4, 8), (8, 16)):
                nc.sync.dma_start(
                    out=wqk_sb[:, lo:hi, :], in_=wqk_v[:, lo:hi, :]
                )
                nc.sync.dma_start(
                    out=xt0[:, lo:hi, :], in_=x8_v[:, lo:hi, 0:TB]
                )
                nc.scalar.dma_start(
                    out=dwqk_sb[:, lo:hi, :], in_=dwqk_v[:, lo:hi, :]
                )
                nc.scalar.dma_start(
                    out=dxt0[:, lo:hi, :], in_=dx8_v[:, lo:hi, 0:TB]
                )

            cos_sb = wpool.tile([128, T], BF16, tag="cos")
            sinh_sb = wpool.tile([128, T], BF16, tag="sinh")
            nc.sync.dma_start(out=cos_sb[:, 0:TB], in_=cos_d[:, 0:TB])
            nc.sync.dma_start(out=sinh_sb[:, 0:TB], in_=sinh_d[:, 0:TB])

            def load_x(b, tb):
                t = xp.tile(
                    [128, NMC, TB], FP8, tag=f"x{tb}", name=f"x8_b{b}_t{tb}"
                )
                dt_ = xp.tile(
                    [128, NMC, TB], FP8, tag=f"dx{tb}",
                    name=f"dx8_b{b}_t{tb}",
                )
                x_tiles[(b, tb)] = (t, dt_)
                lo = b * T + tb * TB
                for m0 in range(0, NMC, 8):
                    nc.sync.dma_start(
                        out=t[:, m0 : m0 + 8, :],
                        in_=x8_v[:, m0 : m0 + 8, lo : lo + TB],
                    )
                    nc.sync.dma_start(
                        out=dt_[:, m0 : m0 + 8, :],
                        in_=dx8_v[:, m0 : m0 + 8, lo : lo + TB],
                    )

            # wv rides alongside tb0's QK matmuls; x block 1 follows
            for m0 in range(0, NMC, 8):
                nc.sync.dma_start(
                    out=wv_sb[:, m0 : m0 + 8, :], in_=wv_v[:, m0 : m0 + 8, :]
                )
                nc.scalar.dma_start(
                    out=dwv_sb[:, m0 : m0 + 8, :],
                    in_=dwv_v[:, m0 : m0 + 8, :],
                )
            load_x(0, 1)
            nc.sync.dma_start(out=cos_sb[:, TB:], in_=cos_d[:, TB:])
            nc.sync.dma_start(out=sinh_sb[:, TB:], in_=sinh_d[:, TB:])
            tri01 = wpool.tile([128, 128], BF16, tag="tri01")
            nc.sync.dma_start(out=tri01, in_=tri_d[:, :])
            ident = wpool.tile([128, 128], BF16, tag="ident")
            nc.sync.dma_start(out=ident, in_=ident_d[:, :])
            ones_col = wpool.tile([128, 1], BF16, tag="ones_c")
            nc.vector.memset(ones_col, 1.0)
            ones_row = wpool.tile([1, 128], BF16, tag="ones_r")
            nc.vector.memset(ones_row, 1.0)

            for tb in range(2, NTB_B):
                load_x(0, tb)

            wo_sb = wpool.tile([128, HPC, D], BF16, tag="wo")
            nc.sync.dma_start(out=wo_sb, in_=wo_v)

            for b in range(B):
                t0 = b * T  # global t offset of this batch
                _mark(nc, f"b{b}_proj")

                # persistent per-batch tensors (slots reused across b)
                qT = big.tile([128, HPC, T], BF16, tag="qT")   # [hd, h, t]
                kT = big.tile([128, HPC, T], BF16, tag="kT")
                vN = big.tile([128, NKC, M_PC], BF16, tag="vN")  # [tk, j, n]
                oT = big.tile([128, HPC, T], BF16, tag="oT")   # attn out

                # ---------------- projections + RoPE ----------------
                warm = []  # pre-emitted attention chunks
                with tc.tile_pool(name="qk_ps", bufs=1, space="PSUM") as qkp, \
                     tc.tile_pool(name="v_ps", bufs=1, space="PSUM") as vps:
                    for tb in range(NTB_B):
                        xt, dxt = x_tiles[(b, tb)]
                        ts_l = slice(tb * TB, (tb + 1) * TB)   # in-batch
                        ps = {}
                        for h in range(HPC):
                            for nm in ("q", "k"):
                                ps[nm, h] = qkp.tile(
                                    [128, TB], F32, tag=f"{nm}{h}",
                                    name=f"ps_{nm}{h}",
                                )
                        # 3 error-compensated fp8 DoubleRow streams per
                        # K-pair: x8*W8 + x8*dW8 + dx8*W8 (dx*dW ~0.1%,
                        # dropped)
                        qk_streams = (
                            (wqk_sb, 0), (dwqk_sb, 0), (wqk_sb, 1)
                        )
                        for kp in range(NMC // 2):
                            kps = slice(2 * kp, 2 * kp + 2)
                            for h in range(HPC):
                                for ni, nm in ((0, "q"), (1, "k")):
                                    hs = slice(
                                        ni * M_PC + h * HD,
                                        ni * M_PC + (h + 1) * HD,
                                    )
                                    for si, (wa, use_dx) in enumerate(
                                        qk_streams
                                    ):
                                        nc.tensor.matmul(
                                            ps[nm, h],
                                            lhsT=wa[:, kps, hs],
                                            rhs=(dxt if use_dx else xt)[
                                                :, kps, :
                                            ],
                                            start=(kp == 0 and si == 0),
                                            stop=(
                                                kp == NMC // 2 - 1
                                                and si == 2
                                            ),
                                            perf_mode=(
                                                mybir.MatmulPerfMode.DoubleRow
                                            ),
                                        )
                        # raw bf16 casts early on Act (deps ready now)
                        raws = {}
                        for nm in ("q", "k"):
                            for h in range(HPC):
                                raw = rp.tile(
                                    [128, TB], BF16, tag=f"raw{nm}{h}"
                                )
                                nc.scalar.activation(raw, ps[nm, h], Copy)
                                raws[nm, h] = raw
                        # RoPE before V: the qk/sw PSUM banks drain while the
                        # PE streams V matmuls, so the next phase's bank-WAR
                        # waits resolve before the PE gets there
                        for nm, dest in (("q", qT), ("k", kT)):
                            for h in range(HPC):
                                raw = raws[nm, h]
                                swb = rp.tile([128, TB], BF16, tag="swb")
                                nc.sync.dma_start(
                                    out=swb[0:64, :], in_=raw[64:128, :]
                                )
                                nc.sync.dma_start(
                                    out=swb[64:128, :], in_=raw[0:64, :]
                                )
                                t2 = rp.tile([128, TB], BF16, tag="t2")
                                nc.vector.tensor_mul(t2, swb, sinh_sb[:, ts_l])
                                t1 = rp.tile([128, TB], BF16, tag="t1")
                                nc.vector.tensor_mul(t1, raw, cos_sb[:, ts_l])
                                nc.vector.tensor_add(dest[:, h, ts_l], t1, t2)
                        # V directly in natural layout: per 128-wide tk
                        # chunk, same 3-stream fp8 DoubleRow scheme
                        def v_chunk(s):
                            j = tb * JPG + s
                            pv = vps.tile(
                                [128, M_PC], F32, tag=f"v{s % 2}",
                                name=f"ps_v{s % 2}",
                            )
                            ss = slice(s * 128, (s + 1) * 128)
                            for kp in range(NMC // 2):
                                kps = slice(2 * kp, 2 * kp + 2)
                                for si, (xa, wa) in enumerate(
                                    ((xt, wv_sb), (xt, dwv_sb), (dxt, wv_sb))
                                ):
                                    nc.tensor.matmul(
                                        pv,
                                        lhsT=xa[:, kps, ss],
                                        rhs=wa[:, kps, :],
                                        start=(kp == 0 and si == 0),
                                        stop=(kp == NMC // 2 - 1 and si == 2),
                                        perf_mode=(
                                            mybir.MatmulPerfMode.DoubleRow
                                        ),
                                    )
                            nc.scalar.activation(vN[:, j, :], pv, Copy)

                        for s in range(JPG):
                            # tb1's tail / tb2 / tb3 V chunks are deferred
                            # into the attention chunks (the window is
                            # Act-bound there; these give the PE ~1.7us
                            # each and move their Act/DVE load out of the
                            # backlog)
                            if tb == 0 or (tb == 1 and s < 2):
                                v_chunk(s)
                            # between tb3's V chunks, warm up attention: the
                            # (qg0, j0) score pair + exp run in drained qk
                            # PSUM slots, their latency hidden by V matmuls,
                            # so the attention loop starts with a full
                            # pipeline instead of a fill bubble
                            if tb == 2 and s in (1, 2):
                                # chunk (0,0) after V s1 in the q-tag slots,
                                # chunk (0,1) after V s2 in the k-tag slots:
                                # the attention loop starts two chunks deep,
                                # matching the Z trail exactly
                                wj = 0 if s == 1 else 1
                                woff = 0 if wj == 0 else 128
                                wtag = "q" if s == 1 else "k"
                                wpt2 = asb.tile(
                                    [128, HPC, TB], BF16, tag="pt2",
                                    name="warm_pt",
                                )
                                for h in range(HPC):
                                    wst = qkp.tile(
                                        [128, TB], F32,
                                        tag=f"{wtag}{h}", name="warm_st",
                                    )
                                    nc.tensor.matmul(
                                        wst[:, woff:],
                                        lhsT=kT[:, h, wj * 128 : (wj + 1) * 128],
                                        rhs=qT[:, h, woff:TB],
                                        start=True,
                                        stop=True,
                                    )
                                    nc.scalar.activation(
                                        wpt2[:, h, woff:], wst[:, woff:],
                                        Exp, scale=SCALE_EXP,
                                    )
                                    nc.vector.tensor_mul(
                                        wpt2[:, h, woff : woff + 128],
                                        wpt2[:, h, woff : woff + 128],
                                        tri01,
                                    )
                                warm.append([0, wj, (wpt2, woff), False])



                _mark(nc, f"b{b}_attn")
                # ---------------- attention + out-projection ----------------
                # Both heads' score tiles share one 2-bank PSUM tile so a
                # single exp call covers them (Act is the scarce engine in
                # this window). Z/PV trail the score/exp stream by 2/3
                # chunks, software-pipelined ACROSS q-groups. Z is
                # accumulated as [128,1] columns via transposed matmuls
                # (exp chunk stationary, ones moving) -- ~1 PE row per chunk
                # instead of up-to-512 -- and routed back to row layout off
                # the PE (recip -> PE transpose -> Act copy -> 8-descriptor
                # DMA). The out-projection interleaves with the chunk
                # stream, one 128-column block per chunk once its q-group is
                # normalized, keeping the PE fed while Act drains exps; the
                # leftovers plus the last group's norms form the batch tail
                # on the score-tile ring.
                with tc.tile_pool(name="st_ps", bufs=2, space="PSUM") as stp, \
                     tc.tile_pool(name="pv_ps", bufs=1, space="PSUM") as pvp, \
                     tc.tile_pool(name="z_ps", bufs=1, space="PSUM") as zpp, \
                     tc.tile_pool(name="fo_ps", bufs=1, space="PSUM") as fop:
                    norm_q = []  # pending [qg, h, ready_at_pv_call]
                    zr_rows = {}  # qg -> [1, HPC*JPG, 128] recip-Z row tile
                    zr_pend = []  # [qg, zrcol, z_call_at_emit]
                    ob_q = []  # (tb, nb) out-proj blocks ready to emit
                    fs_state = {}  # tb -> current fs store-group tile
                    n_z = [0]
                    n_pv = [0]

                    def norm_step(pool, tag):
                        nqg, h, _ = norm_q.pop(0)
                        qs0 = nqg * TB
                        zbp = pool.tile([128, TB], F32, tag=tag, name="zbp")
                        nc.tensor.matmul(
                            zbp,
                            lhsT=ones_row,
                            rhs=zr_rows[nqg][0:1, h * JPG : (h + 1) * JPG, :],
                            start=True,
                            stop=True,
                        )
                        nc.vector.tensor_mul(
                            oT[:, h, qs0 : qs0 + TB],
                            oT[:, h, qs0 : qs0 + TB],
                            zbp,
                        )
                        if h == HPC - 1:  # group fully normalized
                            ob_q.extend(
                                (nqg, nb) for nb in range(D // 128)
                            )

                    def emit_block(btb, nb, pool, tag):
                        """One 128-column out-projection block: 2 matmuls
                        into a borrowed PSUM slot, cast into the current
                        4-block store group, DMA when the group completes."""
                        tbs = slice(btb * TB, (btb + 1) * TB)
                        nbs = slice(nb * 128, (nb + 1) * 128)
                        fo = pool.tile([128, TB], F32, tag=tag, name="fo")
                        for m in range(HPC):
                            nc.tensor.matmul(
                                fo,
                                lhsT=wo_sb[:, m, nbs],
                                rhs=oT[:, m, tbs],
                                start=(m == 0),
                                stop=(m == HPC - 1),
                            )
                        if b == B - 1 and btb == NTB_B - 1 and nb >= 12:
                            # shrink the final stores: the last DMA's
                            # latency is the kernel's tail
                            grp = 2 if nb < 14 else 1
                        else:
                            grp = 4
                        if nb % grp == 0:
                            fs_state[btb] = fsb.tile(
                                [128, 4, TB], F16, tag="fs", name="fs"
                            )
                        fs = fs_state[btb]
                        # 9/16 of casts on Act (DVE also carries the norm
                        # muls and zr copies); the kernel's final two casts
                        # go to Act, whose queue is empty at the end
                        on_act = nb % 2 == 0 or nb % 16 == 7 or grp == 1
                        if on_act:
                            nc.scalar.activation(fs[:, nb % grp, :], fo, Copy)
                        else:
                            nc.vector.tensor_copy(fs[:, nb % grp, :], fo)
                        if nb % grp == grp - 1:
                            # stores alternate between the Pool and SP
                            # queues (either alone serializes ~1.6us per
                            # store, which would gate the batch tail); the
                            # kernel's very last store goes to the Act
                            # queue, which is empty at the end
                            last = (
                                b == B - 1
                                and btb == NTB_B - 1
                                and nb == D // 128 - 1
                            )
                            if last:
                                deng = nc.scalar
                            elif (btb * 16 + nb) % 8 < 4:
                                deng = nc.gpsimd
                            else:
                                deng = nc.sync
                            deng.dma_start(
                                out=out_v[
                                    :,
                                    nb - grp + 1 : nb + 1,
                                    t0 + btb * TB : t0 + (btb + 1) * TB,
                                ],
                                in_=fs[:, 0:grp, :],
                            )

                    def process_zr():
                        """Late half of the Z path: PE-transpose the recip-Z
                        columns to [8,128] (riding the out-proj PSUM slot),
                        copy PSUM->SBUF on Act, and DMA the 8 rows into a
                        single [1, 8*128] row for the broadcast matmul."""
                        zqg, zrcol, _ = zr_pend.pop(0)
                        zrT = fop.tile(
                            [HPC * JPG, 128], BF16, tag="foA", name="zrT"
                        )
                        nc.tensor.transpose(zrT, zrcol, ident)
                        zrS = rp.tile([HPC * JPG, 128], BF16, tag="zs8")
                        # DVE, not Act: at the batch tail Act still has the
                        # trailing exps queued, which would delay the DMA
                        nc.vector.tensor_copy(zrS, zrT)
                        zrow_t = rp.tile(
                            [1, HPC * JPG, 128], BF16, tag="zrow"
                        )
                        # idle Pool queue: the SP queue carries x loads and
                        # output stores, which would delay this tiny DMA
                        nc.gpsimd.dma_start(out=zrow_t, in_=zrS)
                        zr_rows[zqg] = zrow_t
                        for h in range(HPC):
                            norm_q.append([zqg, h, n_pv[0] + 5 + 2 * h])

                    # (qg, j) chunk schedule, flattened; (0,0) was
                    # pre-warmed inside the projection scope
                    sched = [
                        (qg, j)
                        for qg in range(NTB_B)
                        for j in range(JPG * (qg + 1))
                    ][2:]
                    po = {}
                    zcol = {}
                    pend = warm  # [qg, j, (pt2, off), z_done]

                    def z_step(pqg, pj, pts):
                        """Z partial sums (trail 2) as transposed matmuls:
                        one [128,1] PSUM column per (head, 128-wide tq sub),
                        accumulated over key chunks. Each column's stop fires
                        on its last causal chunk; the recip covers all 8
                        columns in one DVE op on the group's last chunk."""
                        n_z[0] += 1
                        if zr_pend and n_z[0] - zr_pend[0][2] >= 3:
                            process_zr()
                        base = JPG * pqg
                        if pqg not in zcol:
                            zcol[pqg] = zpp.tile(
                                [128, HPC * JPG], F32, tag="zc", name="zc"
                            )
                        zc = zcol[pqg]
                        pt2, offp = pts
                        sub0 = max(0, pj - base)
                        # all 8 columns share ONE accumulation group (PSUM
                        # groups are tracked per 2KB zero region = the whole
                        # bank): start only on the group's first matmul --
                        # its start bit marks the bank pending-zero, so the
                        # other columns' first writes see zeros -- and stop
                        # only on its very last
                        for h in range(HPC):
                            for s in range(sub0, JPG):
                                nc.tensor.matmul(
                                    zc[:, h * JPG + s : h * JPG + s + 1],
                                    lhsT=pt2[:, h, s * 128 : (s + 1) * 128],
                                    rhs=ones_col,
                                    start=(pj == 0 and h == 0 and s == 0),
                                    stop=(
                                        pj == JPG * (pqg + 1) - 1
                                        and h == HPC - 1
                                        and s == JPG - 1
                                    ),
                                )
                        if pj == JPG * (pqg + 1) - 1:
                            zrcol = rp.tile(
                                [128, HPC * JPG], BF16, tag="zrc"
                            )
                            with nc.allow_low_precision(
                                reason="bf16 1/Z: 0.4% rel, in tolerance"
                            ):
                                nc.vector.reciprocal(zrcol, zc)
                            zr_pend.append([pqg, zrcol, n_z[0]])

                    def pv_step(pqg, pj, pts):
                        """PV matmuls (trail 3); one paced 1/Z broadcast may
                        ride along per call once its DMA has had >= 5 chunks
                        to land. Returns True if a broadcast was emitted."""
                        n_pv[0] += 1
                        popped = False
                        if norm_q and norm_q[0][2] <= n_pv[0]:
                            norm_step(fop, "foA")
                            popped = True
                        pjmax = JPG * (pqg + 1)
                        if (pqg, 0) not in po:
                            for h in range(HPC):
                                po[pqg, h] = pvp.tile(
                                    [128, TB], F32, tag=f"po{h}", name=f"po{h}"
                                )
                        pt2, offp = pts
                        last_c = pj == pjmax - 1
                        qs0 = pqg * TB
                        # on the group's last chunk, finish h1 first and
                        # emit its (DVE) oT copy immediately -- the next
                        # group's po1 bank reuse waits on it; h0's copy
                        # rides Act
                        for h in ((1, 0) if last_c else (0, 1)):
                            nc.tensor.matmul(
                                po[pqg, h][:, offp:],
                                lhsT=vN[:, pj, h * HD : (h + 1) * HD],
                                rhs=pt2[:, h, offp:],
                                start=(pj == 0),
                                stop=last_c,
                            )
                            if last_c:
                                if h == 0:
                                    nc.scalar.activation(
                                        oT[:, h, qs0 : qs0 + TB],
                                        po[pqg, h],
                                        Copy,
                                    )
                                else:
                                    nc.vector.tensor_copy(
                                        oT[:, h, qs0 : qs0 + TB], po[pqg, h]
                                    )
                        return popped

                    # deferred V chunks: tb2's early (its vN rows are needed
                    # by group 2's PV), tb3's late; each takes the first
                    # conflict-free foA slot at/after its target chunk
                    defer_v = [(NTB_B - 2, s, (0, 1, 5, 9)[s]) for s in range(JPG)]
                    defer_v += [(1, 2, 3), (1, 3, 7)]
                    defer_v += [(NTB_B - 1, s, (13, 17, 21, 25)[s]) for s in range(JPG)]
                    defer_v.sort(key=lambda e: e[2])

                    def emit_v(vtb, s):
                        jv = vtb * JPG + s
                        x8s, dx8s = x_tiles[(b, vtb)]
                        pv = fop.tile(
                            [128, M_PC], F32, tag="foA", name="ps_vd"
                        )
                        ss = slice(s * 128, (s + 1) * 128)
                        for kp in range(NMC // 2):
                            kps = slice(2 * kp, 2 * kp + 2)
                            for si, (xa, wa) in enumerate(
                                ((x8s, wv_sb), (x8s, dwv_sb), (dx8s, wv_sb))
                            ):
                                nc.tensor.matmul(
                                    pv,
                                    lhsT=xa[:, kps, ss],
                                    rhs=wa[:, kps, :],
                                    start=(kp == 0 and si == 0),
                                    stop=(kp == NMC // 2 - 1 and si == 2),
                                    perf_mode=mybir.MatmulPerfMode.DoubleRow,
                                )
                        nc.vector.tensor_copy(vN[:, jv, :], pv)

                    for n_ch, (qg, j) in enumerate(sched):
                        qs0 = qg * TB
                        off = max(0, (j - JPG * qg) * 128)
                        ks = slice(j * 128, (j + 1) * 128)
                        st2 = stp.tile([128, HPC, TB], F32, tag="st2")
                        for h in range(HPC):
                            nc.tensor.matmul(
                                st2[:, h, off:],
                                lhsT=kT[:, h, ks],
                                rhs=qT[:, h, qs0 + off : qs0 + TB],
                                start=True,
                                stop=True,
                            )
                        # exp first (it's on the PE's st2-ring critical
                        # path), then zero the diagonal block's masked wedge
                        # on the SBUF side where the 2-chunk Z/PV trail
                        # hides the DVE latency
                        pt2 = asb.tile([128, HPC, TB], BF16, tag="pt2")
                        nc.scalar.activation(
                            pt2[:, :, off:], st2[:, :, off:], Exp, scale=SCALE_EXP
                        )
                        pend.append([qg, j, (pt2, off), False])
                        popped = False
                        if len(pend) >= 5:
                            it = pend.pop(0)
                            popped = pv_step(it[0], it[1], it[2])
                        # masked-wedge zeroing emitted after pv_step: the
                        # group-end oT copies get DVE queue priority (the
                        # wedge isn't read until the Z step 2 chunks out)
                        if j >= JPG * qg:
                            for h in range(HPC):
                                nc.vector.tensor_mul(
                                    pt2[:, h, off : off + 128],
                                    pt2[:, h, off : off + 128],
                                    tri01,
                                )
                        if (
                            defer_v
                            and n_ch >= defer_v[0][2]
                            and not popped
                            and not zr_pend
                        ):
                            vtb, vs, _ = defer_v.pop(0)
                            emit_v(vtb, vs)
                        if b == 0 and n_ch == 10:
                            # b1's x loads: emitted once b0's deferred V
                            # matmuls (the last xt readers) are in the stream
                            for ltb in range(NTB_B):
                                load_x(1, ltb)
                        if len(pend) >= 3:
                            it = pend[-3]
                            z_step(it[0], it[1], it[2])
                            it[3] = True
                    for it in pend:
                        if not it[3]:
                            z_step(it[0], it[1], it[2])
                    while pend:
                        it = pend.pop(0)
                        pv_step(it[0], it[1], it[2])
                    # batch tail: all remaining out-proj blocks, rotating
                    # over five idle PSUM slots so each block's WAR lands
                    # on a cast several blocks back. A few blocks lead the
                    # last group's Z transpose (hiding the recip's DVE
                    # queue), and more cover its zr DMA before the norms.
                    rot = [
                        (stp, "st2"),
                        (pvp, "po0"),
                        (pvp, "po1"),
                        (fop, "foA"),
                        (zpp, "zc"),
                    ]
                    r = [0]

                    def tail_slot():
                        p, tg = rot[r[0] % len(rot)]
                        r[0] += 1
                        return p, tg

                    k = 0
                    while ob_q and k < 3:
                        emit_block(*ob_q.pop(0), *tail_slot())
                        k += 1
                    while zr_pend:
                        process_zr()
                    _mark(nc, f"b{b}_outproj")
                    first = True
                    while ob_q or norm_q:
                        k = 0
                        while ob_q and k < (8 if first else 4):
                            emit_block(*ob_q.pop(0), *tail_slot())
                            k += 1
                        first = False
                        if norm_q:
                            norm_step(*tail_slot())
    _legalize_waits(nc)
    return nc


_NC_CACHE = None


def _get_program():
    global _NC_CACHE
    if _NC_CACHE is None:
        _NC_CACHE = build_program()
    return _NC_CACHE


def _rope_tables():
    inv_freq = 1.0 / (ROPE_THETA ** (np.arange(0, HD, 2, dtype=np.float32) / HD))
    freqs = np.arange(T, dtype=np.float32)[:, None] * inv_freq[None, :]  # (T, 64)
    emb = np.concatenate([freqs, freqs], axis=-1)                        # (T, 128)
    cosT = np.ascontiguousarray(np.cos(emb).T).astype(BF16_NP)           # [128, T]
    sinT = np.sin(emb).T.astype(np.float32)
    sinhT = np.ascontiguousarray(
        np.concatenate([-sinT[: HD // 2], sinT[HD // 2 :]], axis=0)
    ).astype(BF16_NP)
    return cosT, sinhT


def _fp8_pair(a):
    """Quantize to e4m3 value + residual (a ~= hi + lo, err ~0.1-0.3%)."""
    hi = np.ascontiguousarray(a).astype(FP8_NP)
    lo = np.ascontiguousarray(a - hi.astype(np.float32)).astype(FP8_NP)
    return hi, lo


def kernel(x, Wq, Wk, Wv, Wo, **run_kwargs):
    x = np.asarray(x, dtype=np.float32)
    Wq = np.asarray(Wq, dtype=np.float32)
    Wk = np.asarray(Wk, dtype=np.float32)
    Wv = np.asarray(Wv, dtype=np.float32)
    Wo = np.asarray(Wo, dtype=np.float32)

    nc = _get_program()
    cosT, sinhT = _rope_tables()
    xT = x.reshape(BT, D).T  # [D, BT]
    x8, dx8 = _fp8_pair(xT)
    # S^T[tk, tq] causal mask for the diagonal block: keep tq(col) >= tk(row)
    r = np.arange(128)
    triM = (r[None, :] >= r[:, None]).astype(BF16_NP)
    identM = np.eye(128, dtype=BF16_NP)

    in_maps = []
    for c in range(NCORES):
        sl = slice(c * M_PC, (c + 1) * M_PC)
        wqk8, dwqk8 = _fp8_pair(
            np.concatenate([Wq[sl, :].T, Wk[sl, :].T], axis=1) * WSCALE
        )
        wv8, dwv8 = _fp8_pair(Wv[sl, :].T * WSCALE)
        in_maps.append(
            {
                "xT8": x8,
                "dxT8": dx8,
                "triT": triM,
                "identT": identM,
                "wqk8T": wqk8,
                "dwqk8T": dwqk8,
                "wv8T": wv8,
                "dwv8T": dwv8,
                "woT": np.ascontiguousarray(Wo[:, sl].T).astype(BF16_NP),
                "cosT": cosT,
                "sinhT": sinhT,
            }
        )

    res = run_bass_kernel_spmd(nc, in_maps, list(range(NCORES)), **run_kwargs)
    acc = np.zeros((D, BT), dtype=np.float32)
    for c in range(NCORES):
        acc += res.results[c]["partialT"].astype(np.float32)
    # V rides the WSCALE weight pre-scale through oT and Wo; undo it here
    out = np.ascontiguousarray(acc.T).reshape(B, T, D) * (1.0 / WSCALE)
    if run_kwargs:
        return out, res
    return out

